# revision 24
# baseline (speedup 1.0000x reference)
"""Trainium2 Bass kernel for nn_DecoderBlock (B=4, T=S=1024, E=1024, H=16).

v5: startup, LN2-bridge, and tail improvements on the v3 attention
restructure.

Sharding: 8-way, zero-collective. Core c handles batch b=c//2; parity
p=c%2 selects the query-block set (128-row blocks of the sequence):
p=0 -> blocks {0,1,6,7}, p=1 -> blocks {2,3,4,5}. Queries are stored
grouped as [G0: 256 | G1: 256] columns. This balances the anti-causal
mask (query q attends keys k >= q) so every core runs the identical
instruction stream (SPMD) while skipping fully-masked score blocks:
G0 processes key chunks 0..7, G1 only 4..7 (12 chunk-blocks per head
instead of 16); per-core 0/1 masks (inputs) cover the partially-masked
chunks.

Other points:
  - xT arrives in column slices and the V projection runs first, so the
    first matmul starts ~2us in.
  - LN2 emits bf16 bT_bf directly (one sub + one mult per chunk); the
    FFN residual reads bT_bf, halving the LN2->FFN bridge.
  - FFN-down weights prefetch into their own pool during FFN-up.
  - AV accumulates per group into one PSUM bank [65, 512] per head
    (ones column -> denominator at row 64); softmax normalization is
    a batched reciprocal every 4 heads + one selector matmul and one
    [128,512] DVE multiply per head pair.
  - FFN relu on DVE; FFN-down weights prefetched into their own pool.
  - Weight DMAs are sliced so first consumers start early.
"""
import numpy as np

import ml_dtypes
import concourse.bacc as bacc
import concourse.mybir as mybir
import concourse.tile as tile
from concourse.alu_op_type import AluOpType
from concourse.bass_utils import run_bass_kernel_spmd

P = 128
E = 1024
T = 1024
S = 1024
NQ = 512          # query rows per core
NG = 256          # query rows per group
H4 = 4096         # FFN hidden
KC = 8            # E / P chunks
KH = 32           # H4 / P chunks
VW = 65           # V columns per head incl. ones column
F32 = mybir.dt.float32
BF16 = mybir.dt.bfloat16
AF = mybir.ActivationFunctionType
EXP_SCALE = 1.0 / 8.0   # 1/sqrt(head_size)
EPS = 1e-5

_NC_CACHE = None


def build_nc():
    nc = bacc.Bacc("TRN2", target_bir_lowering=False, debug=False)

    def din(n, s, dt=BF16):
        return nc.dram_tensor(n, s, dt, kind="ExternalInput").ap()

    xT = din("xT", [P, KC, T])               # target^T, pre-tiled
    xTq_bf = din("xTq_bf", [P, KC, NQ])      # query slice bf16 (grouped)
    xTq_f = din("xTq_f", [P, KC, NQ], F32)   # query slice f32 (residual)
    eT = din("eT", [P, KC, S])               # enc_src^T
    m01 = din("m01", [P, 4, NQ])             # keep-mask: [:, :, 0:256] = G0
    #                                          chunks 0-3, [:, :, 256:] = G1
    #                                          chunks 4-7
    sel4 = din("sel4", [P, 256])             # denominator broadcast selectors
    wd = {n: din(n, [P, KC, E]) for n in
          ("wq", "wk", "wv", "wo", "cq", "ck", "cv", "co")}
    w1 = din("w1", [4, P, KC, 1024])         # FFN up, col-groups of 1024
    w2 = din("w2", [4, P, KH, 256])          # FFN down, col-groups of 256
    outT = nc.dram_tensor("outT", [P, KC, NQ], F32, kind="ExternalOutput").ap()

    with tile.TileContext(nc) as tc:
        # ---- constants ----
        const = tc.alloc_tile_pool(name="const", bufs=1)
        ones_f = const.tile([P, 1], F32)
        nc.vector.memset(ones_f[:], 1.0)
        ones_bf = const.tile([P, 1], BF16)
        nc.scalar.activation(ones_bf[:], ones_f[:], AF.Copy)
        eps_t = const.tile([P, 1], F32)
        nc.vector.memset(eps_t[:], EPS)
        ones_row = const.tile([1, P], BF16)
        nc.vector.memset(ones_row[:], 1.0)
        # denominator broadcast selectors (DMA'd, not memset — avoids
        # scheduler-ordering hazards). cols 0:128 = even-pair pattern
        # (partition 0 -> out 0-63, partition 32 -> out 64-127); cols
        # 128:256 = odd-pair pattern (64 -> 0-63, 96 -> 64-127).
        sel_sb = const.tile([P, 256], BF16)
        nc.sync.dma_start(sel_sb[:], sel4)

        # ---- PSUM pools (exactly 8 banks: 4 + 2 + 2) ----
        psc = tc.alloc_tile_pool(name="psc", bufs=2, space="PSUM")   # 2-bank tiles
        pav = tc.alloc_tile_pool(name="pav", bufs=2, space="PSUM")   # [P,NQ]
        pstat = tc.alloc_tile_pool(name="pstat", bufs=1, space="PSUM")  # 2x [1,NQ]

        # ---- small SBUF pools ----
        rows = tc.alloc_tile_pool(name="rows", bufs=1)   # [8,NQ]/[1,NQ] stats
        bc = tc.alloc_tile_pool(name="bc", bufs=2)       # [P,NQ] recip/broadcast
        sqp = tc.alloc_tile_pool(name="sqp", bufs=2)     # LN bf16 shadows

        # ---- left side: long-lived activations ----
        xtqp = tc.alloc_tile_pool(name="xtqp", bufs=1, side="left")
        xTq = xtqp.tile([P, KC, NQ], F32, name="xTq")    # residual stream
        qtp = tc.alloc_tile_pool(name="qtp", bufs=1, side="left")
        QT = qtp.tile([P, KC, NQ], BF16, name="QT")
        caqp = tc.alloc_tile_pool(name="caqp", bufs=1, side="left")
        caQT = caqp.tile([P, KC, NQ], BF16, name="caQT")
        xbqp = tc.alloc_tile_pool(name="xbq", bufs=1, side="left")
        xTq_b = xbqp.tile([P, KC, NQ], BF16, name="xTq_b")
        otp = tc.alloc_tile_pool(name="otp", bufs=1, side="left")
        OT = otp.tile([P, KC, NQ], BF16, name="OT")

        # ---- right side, bottom of stack: weight ring + persistent inputs --
        ring = tc.alloc_tile_pool(name="ring", bufs=2, side="right")
        maskp = tc.alloc_tile_pool(name="maskp", bufs=1, side="right")
        mask_sb = maskp.tile([P, 4, NQ], BF16, name="mask_sb")

        # ---- phase 1: self-attn K^T / V projections (full seq) ----
        kvp = tc.alloc_tile_pool(name="kv", bufs=1, side="right")
        KT = kvp.tile([P, KC, S], BF16, name="KT")
        Vaug = kvp.tile([P, KC, 16, VW], BF16, name="Vaug")
        nc.vector.memset(Vaug[:, :, :, 64:65], 1.0)
        xmat = tc.alloc_tile_pool(name="xmat", bufs=1, side="right")
        xT_sb = xmat.tile([P, KC, T], BF16, name="xT_sb")

        # kick off input DMAs (order = DMA queue order). xT arrives in
        # column slices so the V projection (sc-outer) starts after the
        # first 256KB.
        wv = ring.tile([P, KC, E], BF16, name="w")
        nc.sync.dma_start(xT_sb[:, :, 0:128], xT[:, :, 0:128])
        nc.sync.dma_start(wv[:, 0:2, 0:512], wd["wv"][:, 0:2, 0:512])
        nc.sync.dma_start(xT_sb[:, :, 128:512], xT[:, :, 128:512])
        nc.sync.dma_start(wv[:, 2:8, 0:512], wd["wv"][:, 2:8, 0:512])
        nc.sync.dma_start(xT_sb[:, :, 512:1024], xT[:, :, 512:1024])
        nc.sync.dma_start(wv[:, :, 512:1024], wd["wv"][:, :, 512:1024])
        wk = ring.tile([P, KC, E], BF16, name="w")
        nc.sync.dma_start(wk[:, :, 0:512], wd["wk"][:, :, 0:512])
        nc.sync.dma_start(wk[:, :, 512:1024], wd["wk"][:, :, 512:1024])
        nc.sync.dma_start(xTq_b[:], xTq_bf)
        nc.sync.dma_start(xTq[:], xTq_f)

        # V: keys on partitions, per-head augmented slots
        for sc in range(KC):
            pair = psc.tile([P, 2, NQ], F32, name="ps")
            for nt in range(2):
                for kc in range(KC):
                    nc.tensor.matmul(pair[:, nt, :],
                                     xT_sb[:, kc, sc * P:(sc + 1) * P],
                                     wv[:, kc, nt * NQ:(nt + 1) * NQ],
                                     start=(kc == 0), stop=(kc == KC - 1))
            for nt in range(2):
                nc.scalar.activation(
                    Vaug[:, sc, nt * 8:(nt + 1) * 8, 0:64],
                    pair[:, nt, :].rearrange("p (h w) -> p h w", w=64), AF.Copy)
        # K^T: kc-outer over 4-psum groups
        for nt in range(2):
            for g in range(2):
                pair = [psc.tile([P, 2, NQ], F32, name="ps") for _ in range(2)]
                for kc in range(KC):
                    for mi in range(4):
                        m = 4 * g + mi
                        nc.tensor.matmul(pair[mi // 2][:, mi % 2, :],
                                         wk[:, kc, m * P:(m + 1) * P],
                                         xT_sb[:, kc, nt * NQ:(nt + 1) * NQ],
                                         start=(kc == 0), stop=(kc == KC - 1))
                for mi in range(4):
                    m = 4 * g + mi
                    nc.scalar.activation(KT[:, m, nt * NQ:(nt + 1) * NQ],
                                         pair[mi // 2][:, mi % 2, :], AF.Copy)

        xmat.release()

        # ---- phase 2: Q^T projections (self + cross) ----
        wq = ring.tile([P, KC, E], BF16, name="w")
        nc.sync.dma_start(wq[:, :, 0:512], wd["wq"][:, :, 0:512])
        nc.sync.dma_start(wq[:, :, 512:1024], wd["wq"][:, :, 512:1024])
        cq = ring.tile([P, KC, E], BF16, name="w")
        nc.sync.dma_start(cq[:, :, 0:512], wd["cq"][:, :, 0:512])
        nc.sync.dma_start(cq[:, :, 512:1024], wd["cq"][:, :, 512:1024])
        nc.sync.dma_start(mask_sb[:], m01)

        for src, dst in ((wq, QT), (cq, caQT)):
            for mp in range(4):
                pair = psc.tile([P, 2, NQ], F32, name="ps")
                for mi in range(2):
                    m = 2 * mp + mi
                    for kc in range(KC):
                        nc.tensor.matmul(pair[:, mi, :],
                                         src[:, kc, m * P:(m + 1) * P],
                                         xTq_b[:, kc, :],
                                         start=(kc == 0), stop=(kc == KC - 1))
                for mi in range(2):
                    nc.scalar.activation(dst[:, 2 * mp + mi, :],
                                         pair[:, mi, :], AF.Copy)

        # ---- attention helper ----
        # Per group g: query cols [NG*g, NG*g+NG); key chunks c0..c0+nch.
        # masked groups apply the per-core 0/1 mask to their first mask_n
        # chunks (host guarantees all later chunks are unmasked).
        def attention(QTl, KTl, Vaugl, OTl, masked):
            expp = tc.alloc_tile_pool(name="expp", bufs=8, side="right")
            if masked:
                groups = ((0, 0, 8, True), (1, 4, 4, True))  # (g, c0, nch, msk)
            else:
                groups = ((0, 0, 8, False), (1, 0, 8, False))

            D = None
            for hp in range(8):   # head pairs: A=2hp (rows 0-63), B (64-127)
                if hp % 2 == 0:
                    # fresh denominator tile per 4-head batch; rows other
                    # than 0/32/64/96 stay 1.0 so the reciprocal + selector
                    # matmul read benign values.
                    D = bc.tile([P, NQ], F32, name="D")
                    nc.vector.memset(D[:], 1.0)
                exs = {0: [], 1: []}   # per h2: list of (g, ex_tile, s0, sn)
                for g, c0, nch, msk in groups:
                    q0 = NG * g
                    for o in range(0, nch, 4):
                        s0, sn = c0 + o, min(4, nch - o)
                        ps = [psc.tile([P, 4, NG], F32, name="ps")
                              for _ in range(2)]
                        for j in range(sn):
                            kc = s0 + j
                            for h2 in range(2):
                                lo = 64 * h2
                                nc.tensor.matmul(
                                    ps[h2][:, j, :],
                                    KTl[lo:lo + 64, hp, kc * P:(kc + 1) * P],
                                    QTl[lo:lo + 64, hp, q0:q0 + NG],
                                    start=True, stop=True)
                        for h2 in range(2):
                            ex = expp.tile([P, 4, NG], BF16, name="ex")
                            nc.scalar.activation(ex[:, 0:sn, :],
                                                 ps[h2][:, 0:sn, :],
                                                 AF.Exp, scale=EXP_SCALE)
                            if msk and o == 0:
                                nc.vector.tensor_tensor(
                                    ex[:, 0:sn, :], ex[:, 0:sn, :],
                                    mask_sb[:, 0:sn, q0:q0 + NG],
                                    AluOpType.mult)
                            exs[h2].append((g, ex, s0, sn))
                # AV per head: accumulate each group into its half of one
                # PSUM bank [65, 512]; row 64 = softmax denominator.
                for h2 in range(2):
                    h = 2 * hp + h2
                    lo = 64 * h2
                    pavt = pav.tile([P, NQ], F32, name="pav")
                    for g, c0, nch, msk in groups:
                        q0 = NG * g
                        tl = [t for t in exs[h2] if t[0] == g]
                        n_mm = sum(t[3] for t in tl)
                        i = 0
                        for _, ex, s0, sn in tl:
                            for j in range(sn):
                                nc.tensor.matmul(
                                    pavt[0:VW, q0:q0 + NG],
                                    Vaugl[:, s0 + j, h, :],
                                    ex[:, j, :],
                                    start=(i == 0), stop=(i == n_mm - 1))
                                i += 1
                    # values -> OT (DVE cast), denominator row -> D (scalar)
                    nc.vector.tensor_copy(OTl[lo:lo + 64, hp, :],
                                          pavt[0:64, :])
                    dp = 32 * (h % 4)
                    nc.scalar.activation(D[dp:dp + 1, :], pavt[64:65, :],
                                         AF.Copy)
                # after every 2 pairs (4 heads): reciprocal + apply
                if hp % 2 == 1:
                    Rf = bc.tile([P, NQ], F32, name="Rf")
                    nc.vector.reciprocal_approx_fast(Rf[:], D[:])
                    R8 = bc.tile([P, NQ], BF16, name="R8")
                    nc.vector.tensor_copy(R8[:], Rf[:])
                    for pi, hpx in enumerate((hp - 1, hp)):
                        Bp = pav.tile([P, NQ], F32, name="pav")
                        nc.tensor.matmul(Bp[:],
                                         sel_sb[:, 128 * pi:128 * pi + 128],
                                         R8[:], start=True, stop=True)
                        nc.vector.tensor_tensor(OTl[:, hpx, :],
                                                OTl[:, hpx, :],
                                                Bp[:], AluOpType.mult)
            expp.release()

        # ---- phase 3: self-attention ----
        attention(QT, KT, Vaug, OT, masked=True)
        kvp.release()
        maskp.release()

        # ---- phase 4a: self out-proj + residual into xTq ----
        emat = tc.alloc_tile_pool(name="emat", bufs=1, side="right")
        eT_sb = emat.tile([P, KC, S], BF16, name="eT_sb")
        nc.sync.dma_start(eT_sb[:], eT)
        wo = ring.tile([P, KC, E], BF16, name="w")
        nc.sync.dma_start(wo[:], wd["wo"])
        for mp in range(4):
            pair = psc.tile([P, 2, NQ], F32, name="ps")
            for mi in range(2):
                m = 2 * mp + mi
                for kc in range(KC):
                    nc.tensor.matmul(pair[:, mi, :],
                                     wo[:, kc, m * P:(m + 1) * P],
                                     OT[:, kc, :],
                                     start=(kc == 0), stop=(kc == KC - 1))
            for mi in range(2):
                m = 2 * mp + mi
                nc.vector.tensor_tensor(xTq[:, m, :], pair[:, mi, :],
                                        xTq[:, m, :], AluOpType.add)

        # ---- layernorm helper (in place over X [P,KC,NQ] f32) ----
        def layernorm(X, out_bf=None):
            sum_ps = pstat.tile([1, NQ], F32, name="s1")
            sq_ps = pstat.tile([1, NQ], F32, name="s2")
            for m in range(KC):
                xb = sqp.tile([P, NQ], BF16, name="xb")
                sq = sqp.tile([P, NQ], BF16, name="sq")
                nc.scalar.activation(xb[:], X[:, m, :], AF.Copy)
                nc.vector.tensor_tensor(sq[:], xb[:], xb[:], AluOpType.mult)
                nc.tensor.matmul(sum_ps[:], ones_bf[:], xb[:],
                                 start=(m == 0), stop=(m == KC - 1))
                nc.tensor.matmul(sq_ps[:], ones_bf[:], sq[:],
                                 start=(m == 0), stop=(m == KC - 1))
            mu = rows.tile([1, NQ], F32, name="mu")
            ex2 = rows.tile([1, NQ], F32, name="ex2")
            var = rows.tile([1, NQ], F32, name="var")
            std = rows.tile([1, NQ], F32, name="std")
            rstd = rows.tile([1, NQ], F32, name="rstd")
            nc.scalar.activation(mu[:], sum_ps[:], AF.Copy, scale=1.0 / E)
            nc.scalar.activation(ex2[:], sq_ps[:], AF.Copy, scale=1.0 / E)
            nc.vector.tensor_tensor(var[:], mu[:], mu[:], AluOpType.mult)
            nc.vector.tensor_tensor(var[:], ex2[:], var[:], AluOpType.subtract)
            # rstd = exp(-0.5*ln(var+eps)): keeps the exp table set resident
            # (sqrt lives in a different set -> ~2.7us table switch per LN)
            nc.scalar.activation(std[:], var[:], AF.Ln, bias=eps_t[0:1, :])
            nc.scalar.activation(rstd[:], std[:], AF.Exp, scale=-0.5)
            mub = rows.tile([1, NQ], BF16, name="mub")
            rsb = rows.tile([1, NQ], BF16, name="rsb")
            nc.scalar.activation(mub[:], mu[:], AF.Copy)
            nc.scalar.activation(rsb[:], rstd[:], AF.Copy)
            muB = pav.tile([P, NQ], F32, name="pav")
            nc.tensor.matmul(muB[:], ones_row[:], mub[:], start=True, stop=True)
            rsB = pav.tile([P, NQ], F32, name="pav")
            nc.tensor.matmul(rsB[:], ones_row[:], rsb[:], start=True, stop=True)
            for m in range(KC):
                nc.vector.tensor_tensor(X[:, m, :], X[:, m, :], muB[:],
                                        AluOpType.subtract)
                nc.vector.tensor_tensor(X[:, m, :], X[:, m, :], rsB[:],
                                        AluOpType.mult)
                if out_bf is not None:
                    nc.scalar.activation(out_bf[:, m, :], X[:, m, :], AF.Copy)

        def layernorm_bf(X, out_bf):
            sum_ps = pstat.tile([1, NQ], F32, name="s1")
            sq_ps = pstat.tile([1, NQ], F32, name="s2")
            for m in range(KC):
                xb = sqp.tile([P, NQ], BF16, name="xb")
                sq = sqp.tile([P, NQ], BF16, name="sq")
                nc.scalar.activation(xb[:], X[:, m, :], AF.Copy)
                nc.vector.tensor_tensor(sq[:], xb[:], xb[:], AluOpType.mult)
                nc.tensor.matmul(sum_ps[:], ones_bf[:], xb[:],
                                 start=(m == 0), stop=(m == KC - 1))
                nc.tensor.matmul(sq_ps[:], ones_bf[:], sq[:],
                                 start=(m == 0), stop=(m == KC - 1))
            mu = rows.tile([1, NQ], F32, name="mu")
            ex2 = rows.tile([1, NQ], F32, name="ex2")
            var = rows.tile([1, NQ], F32, name="var")
            std = rows.tile([1, NQ], F32, name="std")
            rstd = rows.tile([1, NQ], F32, name="rstd")
            nc.scalar.activation(mu[:], sum_ps[:], AF.Copy, scale=1.0 / E)
            nc.scalar.activation(ex2[:], sq_ps[:], AF.Copy, scale=1.0 / E)
            nc.vector.tensor_tensor(var[:], mu[:], mu[:], AluOpType.mult)
            nc.vector.tensor_tensor(var[:], ex2[:], var[:], AluOpType.subtract)
            # rstd = exp(-0.5*ln(var+eps)): keeps the exp table set resident
            # (sqrt lives in a different set -> ~2.7us table switch per LN)
            nc.scalar.activation(std[:], var[:], AF.Ln, bias=eps_t[0:1, :])
            nc.scalar.activation(rstd[:], std[:], AF.Exp, scale=-0.5)
            mub = rows.tile([1, NQ], BF16, name="mub")
            rsb = rows.tile([1, NQ], BF16, name="rsb")
            nc.scalar.activation(mub[:], mu[:], AF.Copy)
            nc.scalar.activation(rsb[:], rstd[:], AF.Copy)
            muB = pav.tile([P, NQ], F32, name="pav")
            nc.tensor.matmul(muB[:], ones_row[:], mub[:], start=True, stop=True)
            rsB = pav.tile([P, NQ], F32, name="pav")
            nc.tensor.matmul(rsB[:], ones_row[:], rsb[:], start=True, stop=True)
            rs_sb = bc.tile([P, NQ], BF16, name="rs_sb")
            nc.vector.tensor_copy(rs_sb[:], rsB[:])
            for m in range(KC):
                t = sqp.tile([P, NQ], BF16, name="t")
                nc.vector.tensor_tensor(t[:], X[:, m, :], muB[:],
                                        AluOpType.subtract)
                nc.vector.tensor_tensor(out_bf[:, m, :], t[:], rs_sb[:],
                                        AluOpType.mult)

        # ---- LN1 (xTq -> v^T); overlaps CA K/V projections ----
        layernorm(xTq)

        # ---- phase 5: cross-attn K^T / V projections from eT ----
        ck = ring.tile([P, KC, E], BF16, name="w")
        nc.sync.dma_start(ck[:], wd["ck"])
        cv = ring.tile([P, KC, E], BF16, name="w")
        nc.sync.dma_start(cv[:], wd["cv"])
        cakv = tc.alloc_tile_pool(name="cakv", bufs=1, side="right")
        caKT = cakv.tile([P, KC, S], BF16, name="caKT")
        caVaug = cakv.tile([P, KC, 16, VW], BF16, name="caVaug")
        nc.vector.memset(caVaug[:, :, :, 64:65], 1.0)

        for nt in range(2):
            for g in range(2):
                pair = [psc.tile([P, 2, NQ], F32, name="ps") for _ in range(2)]
                for kc in range(KC):
                    for mi in range(4):
                        m = 4 * g + mi
                        nc.tensor.matmul(pair[mi // 2][:, mi % 2, :],
                                         ck[:, kc, m * P:(m + 1) * P],
                                         eT_sb[:, kc, nt * NQ:(nt + 1) * NQ],
                                         start=(kc == 0), stop=(kc == KC - 1))
                for mi in range(4):
                    m = 4 * g + mi
                    nc.scalar.activation(caKT[:, m, nt * NQ:(nt + 1) * NQ],
                                         pair[mi // 2][:, mi % 2, :], AF.Copy)
        for sc in range(KC):
            pair = psc.tile([P, 2, NQ], F32, name="ps")
            for nt in range(2):
                for kc in range(KC):
                    nc.tensor.matmul(pair[:, nt, :],
                                     eT_sb[:, kc, sc * P:(sc + 1) * P],
                                     cv[:, kc, nt * NQ:(nt + 1) * NQ],
                                     start=(kc == 0), stop=(kc == KC - 1))
            for nt in range(2):
                nc.scalar.activation(
                    caVaug[:, sc, nt * 8:(nt + 1) * 8, 0:64],
                    pair[:, nt, :].rearrange("p (h w) -> p h w", w=64), AF.Copy)

        # ---- phase 6: cross-attention ----
        attention(caQT, caKT, caVaug, OT, masked=False)
        cakv.release()
        emat.release()

        # ---- phase 7: cross out-proj + residual (v^T in xTq) + LN2 ----
        co = ring.tile([P, KC, E], BF16, name="w")
        nc.sync.dma_start(co[:], wd["co"])
        btfp = tc.alloc_tile_pool(name="btf", bufs=1, side="left")
        bT_bf = btfp.tile([P, KC, NQ], BF16, name="bT_bf")
        for mp in range(4):
            pair = psc.tile([P, 2, NQ], F32, name="ps")
            for mi in range(2):
                m = 2 * mp + mi
                for kc in range(KC):
                    nc.tensor.matmul(pair[:, mi, :],
                                     co[:, kc, m * P:(m + 1) * P],
                                     OT[:, kc, :],
                                     start=(kc == 0), stop=(kc == KC - 1))
            for mi in range(2):
                m = 2 * mp + mi
                nc.vector.tensor_tensor(xTq[:, m, :], pair[:, mi, :],
                                        xTq[:, m, :], AluOpType.add)
        layernorm_bf(xTq, bT_bf)   # b^T in bf16; xTq stays un-normalized

        # ---- phase 8: FFN up (relu on DVE); prefetch FFN-down weights ----
        htp = tc.alloc_tile_pool(name="ht", bufs=1, side="right")
        HT = htp.tile([P, KH, NQ], BF16, name="HT")
        wp2 = tc.alloc_tile_pool(name="wp2", bufs=2, side="right")
        for g in range(4):
            w1g = ring.tile([P, KC, 1024], BF16, name="w")
            nc.sync.dma_start(w1g[:, :, 0:512], w1[g][:, :, 0:512])
            nc.sync.dma_start(w1g[:, :, 512:1024], w1[g][:, :, 512:1024])
            for half in range(2):
                pair = [psc.tile([P, 2, NQ], F32, name="ps") for _ in range(2)]
                for kc in range(KC):
                    for mi in range(4):
                        mh = 4 * half + mi
                        nc.tensor.matmul(pair[mi // 2][:, mi % 2, :],
                                         w1g[:, kc, mh * P:(mh + 1) * P],
                                         bT_bf[:, kc, :],
                                         start=(kc == 0), stop=(kc == KC - 1))
                for mi in range(4):
                    mh = 4 * half + mi
                    nc.vector.tensor_scalar_max(HT[:, g * 8 + mh, :],
                                                pair[mi // 2][:, mi % 2, :],
                                                0.0)

        # ---- phase 9: FFN down + residual + LN3 + store ----
        outp = tc.alloc_tile_pool(name="outp", bufs=1, side="left")
        outT_sb = outp.tile([P, KC, NQ], F32, name="outT_sb")
        sum_ps = pstat.tile([1, NQ], F32, name="s1")
        sq_ps = pstat.tile([1, NQ], F32, name="s2")
        for mp in range(4):
            w2g = wp2.tile([P, KH, 256], BF16, name="w2g")
            nc.sync.dma_start(w2g[:], w2[mp])
            pair = psc.tile([P, 2, NQ], F32, name="ps")
            for mi in range(2):
                m = 2 * mp + mi
                for kc2 in range(KH):
                    nc.tensor.matmul(pair[:, mi, :],
                                     w2g[:, kc2, mi * P:(mi + 1) * P],
                                     HT[:, kc2, :],
                                     start=(kc2 == 0), stop=(kc2 == KH - 1))
            for mi in range(2):
                m = 2 * mp + mi
                nc.vector.tensor_tensor(outT_sb[:, m, :], pair[:, mi, :],
                                        bT_bf[:, m, :], AluOpType.add)
                # incremental LN3 stats
                xb = sqp.tile([P, NQ], BF16, name="xb")
                sq = sqp.tile([P, NQ], BF16, name="sq")
                nc.scalar.activation(xb[:], outT_sb[:, m, :], AF.Copy)
                nc.vector.tensor_tensor(sq[:], xb[:], xb[:], AluOpType.mult)
                nc.tensor.matmul(sum_ps[:], ones_bf[:], xb[:],
                                 start=(m == 0), stop=(m == KC - 1))
                nc.tensor.matmul(sq_ps[:], ones_bf[:], sq[:],
                                 start=(m == 0), stop=(m == KC - 1))
        mu = rows.tile([1, NQ], F32, name="mu")
        ex2 = rows.tile([1, NQ], F32, name="ex2")
        var = rows.tile([1, NQ], F32, name="var")
        std = rows.tile([1, NQ], F32, name="std")
        rstd = rows.tile([1, NQ], F32, name="rstd")
        nc.scalar.activation(mu[:], sum_ps[:], AF.Copy, scale=1.0 / E)
        nc.scalar.activation(ex2[:], sq_ps[:], AF.Copy, scale=1.0 / E)
        nc.vector.tensor_tensor(var[:], mu[:], mu[:], AluOpType.mult)
        nc.vector.tensor_tensor(var[:], ex2[:], var[:], AluOpType.subtract)
        nc.scalar.activation(std[:], var[:], AF.Ln, bias=eps_t[0:1, :])
        nc.scalar.activation(rstd[:], std[:], AF.Exp, scale=-0.5)
        mub = rows.tile([1, NQ], BF16, name="mub")
        rsb = rows.tile([1, NQ], BF16, name="rsb")
        nc.scalar.activation(mub[:], mu[:], AF.Copy)
        nc.scalar.activation(rsb[:], rstd[:], AF.Copy)
        muB = pav.tile([P, NQ], F32, name="pav")
        nc.tensor.matmul(muB[:], ones_row[:], mub[:], start=True, stop=True)
        rsB = pav.tile([P, NQ], F32, name="pav")
        nc.tensor.matmul(rsB[:], ones_row[:], rsb[:], start=True, stop=True)
        for m in range(KC):
            nc.vector.tensor_tensor(outT_sb[:, m, :], outT_sb[:, m, :],
                                    muB[:], AluOpType.subtract)
            nc.vector.tensor_tensor(outT_sb[:, m, :], outT_sb[:, m, :],
                                    rsB[:], AluOpType.mult)
            nc.sync.dma_start(outT[:, m, :], outT_sb[:, m, :])

        # right side LIFO
        wp2.release()
        htp.release()
        ring.release()
        # left side LIFO
        outp.release()
        btfp.release()
        otp.release()
        xbqp.release()
        caqp.release()
        qtp.release()
        xtqp.release()
        sqp.release()
        bc.release()
        rows.release()
        pstat.release()
        pav.release()
        psc.release()
        const.release()

    nc.compile()
    return nc


def get_nc():
    global _NC_CACHE
    if _NC_CACHE is None:
        _NC_CACHE = build_nc()
    return _NC_CACHE


def _tile_pc(w):
    """[K*P, N] -> [P, K, N] contiguous chunk-tiled copy."""
    kp, n = w.shape
    return np.ascontiguousarray(
        w.reshape(kp // P, P, n).transpose(1, 0, 2))


def _qblocks(parity):
    return (0, 1, 6, 7) if parity == 0 else (2, 3, 4, 5)


def _qidx(parity):
    return np.concatenate([np.arange(128 * b, 128 * b + 128)
                           for b in _qblocks(parity)])


def make_in_maps(inputs):
    """Build per-core input maps from the full (unsharded) input dict."""
    bf = ml_dtypes.bfloat16
    tgt = np.asarray(inputs['target'], dtype=np.float32)
    enc = np.asarray(inputs['enc_src'], dtype=np.float32)
    wcast = {}
    for src_name, name in (('sa_wq', 'wq'), ('sa_wk', 'wk'), ('sa_wv', 'wv'),
                           ('sa_wo', 'wo'), ('ca_wq', 'cq'), ('ca_wk', 'ck'),
                           ('ca_wv', 'cv'), ('ca_wo', 'co')):
        wcast[name] = _tile_pc(np.asarray(inputs[src_name], np.float32)
                               .astype(bf))
    w1f = np.asarray(inputs['ff_w1'], np.float32).astype(bf)
    wcast['w1'] = np.ascontiguousarray(
        w1f.reshape(KC, P, 4, 1024).transpose(2, 1, 0, 3))
    w2f = np.asarray(inputs['ff_w2'], np.float32).astype(bf)
    wcast['w2'] = np.ascontiguousarray(
        w2f.reshape(KH, P, 4, 256).transpose(2, 1, 0, 3))
    sel = np.zeros((P, 256), np.float32)
    for r in range(4):
        sel[32 * r, 128 * (r // 2) + 64 * (r % 2):
            128 * (r // 2) + 64 * (r % 2) + 64] = 1.0
    wcast['sel4'] = np.ascontiguousarray(sel.astype(bf))
    in_maps = []
    k_loc = np.arange(P, dtype=np.int64)
    for c in range(8):
        b, par = c // 2, c % 2
        qi = _qidx(par)
        # mask: [:, ch, 0:256] covers G0 chunks 0-3 over queries qi[0:256];
        #       [:, ch, 256:512] covers G1 chunks 4-7 over queries qi[256:].
        m = np.zeros((P, 4, NQ), dtype=np.float32)
        for ch in range(4):
            kg0 = 128 * ch + k_loc          # G0 chunk = ch
            kg1 = 128 * (4 + ch) + k_loc    # G1 chunk = 4 + ch
            m[:, ch, 0:NG] = (kg0[:, None] >= qi[None, 0:NG])
            m[:, ch, NG:] = (kg1[:, None] >= qi[None, NG:])
        xTb = np.ascontiguousarray(tgt[b].T)
        xTbq = np.ascontiguousarray(xTb[:, qi])
        mm = {
            'xT': _tile_pc(xTb.astype(bf)),
            'xTq_bf': _tile_pc(xTbq.astype(bf)),
            'xTq_f': _tile_pc(xTbq),
            'eT': _tile_pc(enc[b].T.astype(bf)),
            'm01': np.ascontiguousarray(m.astype(bf)),
        }
        mm.update(wcast)
        in_maps.append(mm)
    return in_maps


def assemble(results):
    out = np.empty((4, T, E), dtype=np.float32)
    for c in range(8):
        b, par = c // 2, c % 2
        qi = _qidx(par)
        o = results[c]['outT']          # [P, KC, NQ]
        out[b, qi, :] = o.transpose(2, 1, 0).reshape(NQ, E)
    return out


def kernel(**inputs):
    nc = get_nc()
    in_maps = make_in_maps(inputs)
    res = run_bass_kernel_spmd(nc, in_maps, core_ids=list(range(8)))
    return assemble(res.results)


# revision 25
# speedup vs baseline: 1.0258x; 1.0258x over previous
"""Trainium2 Bass kernel for nn_DecoderBlock (B=4, T=S=1024, E=1024, H=16).

v5: startup, LN2-bridge, and tail improvements on the v3 attention
restructure.

Sharding: 8-way, zero-collective. Core c handles batch b=c//2; parity
p=c%2 selects the query-block set (128-row blocks of the sequence):
p=0 -> blocks {0,1,6,7}, p=1 -> blocks {2,3,4,5}. Queries are stored
grouped as [G0: 256 | G1: 256] columns. This balances the anti-causal
mask (query q attends keys k >= q) so every core runs the identical
instruction stream (SPMD) while skipping fully-masked score blocks:
G0 processes key chunks 0..7, G1 only 4..7 (12 chunk-blocks per head
instead of 16); per-core 0/1 masks (inputs) cover the partially-masked
chunks.

Other points:
  - xT arrives in column slices and the V projection runs first, so the
    first matmul starts ~2us in.
  - LN2 emits bf16 bT_bf directly (one sub + one mult per chunk); the
    FFN residual reads bT_bf, halving the LN2->FFN bridge.
  - FFN-down weights prefetch into their own pool during FFN-up.
  - AV accumulates per group into one PSUM bank [65, 512] per head
    (ones column -> denominator at row 64); softmax normalization is
    a batched reciprocal every 4 heads + one selector matmul and one
    [128,512] DVE multiply per head pair.
  - FFN relu on DVE; FFN-down weights prefetched into their own pool.
  - Weight DMAs are sliced so first consumers start early.
"""
import numpy as np

import ml_dtypes
import concourse.bacc as bacc
import concourse.mybir as mybir
import concourse.tile as tile
from concourse.alu_op_type import AluOpType
from concourse.bass_utils import run_bass_kernel_spmd

P = 128
E = 1024
T = 1024
S = 1024
NQ = 512          # query rows per core
NG = 256          # query rows per group
H4 = 4096         # FFN hidden
KC = 8            # E / P chunks
KH = 32           # H4 / P chunks
VW = 65           # V columns per head incl. ones column
F32 = mybir.dt.float32
BF16 = mybir.dt.bfloat16
AF = mybir.ActivationFunctionType
EXP_SCALE = 1.0 / 8.0   # 1/sqrt(head_size)
EPS = 1e-5

_NC_CACHE = None


def build_nc():
    nc = bacc.Bacc("TRN2", target_bir_lowering=False, debug=False)

    def din(n, s, dt=BF16):
        return nc.dram_tensor(n, s, dt, kind="ExternalInput").ap()

    xT = din("xT", [P, KC, T])               # target^T, pre-tiled
    xTq_bf = din("xTq_bf", [P, KC, NQ])      # query slice bf16 (grouped)
    xTq_f = din("xTq_f", [P, KC, NQ], F32)   # query slice f32 (residual)
    eT = din("eT", [P, KC, S])               # enc_src^T
    m01 = din("m01", [P, 4, NQ])             # keep-mask: [:, :, 0:256] = G0
    #                                          chunks 0-3, [:, :, 256:] = G1
    #                                          chunks 4-7
    sel4 = din("sel4", [P, 256])             # denominator broadcast selectors
    wd = {n: din(n, [P, KC, E]) for n in
          ("wq", "wk", "wv", "wo", "cq", "ck", "cv", "co")}
    w1 = din("w1", [4, P, KC, 1024])         # FFN up, col-groups of 1024
    w2 = din("w2", [4, P, KH, 256])          # FFN down, col-groups of 256
    outT = nc.dram_tensor("outT", [P, KC, NQ], F32, kind="ExternalOutput").ap()

    with tile.TileContext(nc) as tc:
        # ---- constants ----
        const = tc.alloc_tile_pool(name="const", bufs=1)
        ones_f = const.tile([P, 1], F32)
        nc.vector.memset(ones_f[:], 1.0)
        ones_bf = const.tile([P, 1], BF16)
        nc.scalar.activation(ones_bf[:], ones_f[:], AF.Copy)
        eps_t = const.tile([P, 1], F32)
        nc.vector.memset(eps_t[:], EPS)
        ones_row = const.tile([1, P], BF16)
        nc.vector.memset(ones_row[:], 1.0)
        # denominator broadcast selectors (DMA'd, not memset — avoids
        # scheduler-ordering hazards). cols 0:128 = even-pair pattern
        # (partition 0 -> out 0-63, partition 32 -> out 64-127); cols
        # 128:256 = odd-pair pattern (64 -> 0-63, 96 -> 64-127).
        sel_sb = const.tile([P, 256], BF16)
        nc.sync.dma_start(sel_sb[:], sel4)

        # ---- PSUM pools (exactly 8 banks: 4 + 2 + 2) ----
        psc = tc.alloc_tile_pool(name="psc", bufs=2, space="PSUM")   # 2-bank tiles
        pav = tc.alloc_tile_pool(name="pav", bufs=2, space="PSUM")   # [P,NQ]
        pstat = tc.alloc_tile_pool(name="pstat", bufs=1, space="PSUM")  # 2x [1,NQ]

        # ---- small SBUF pools ----
        rows = tc.alloc_tile_pool(name="rows", bufs=1)   # [8,NQ]/[1,NQ] stats
        bc = tc.alloc_tile_pool(name="bc", bufs=2)       # [P,NQ] recip/broadcast
        sqp = tc.alloc_tile_pool(name="sqp", bufs=2)     # LN bf16 shadows

        # ---- left side: long-lived activations ----
        xtqp = tc.alloc_tile_pool(name="xtqp", bufs=1, side="left")
        xTq = xtqp.tile([P, KC, NQ], F32, name="xTq")    # residual stream
        qtp = tc.alloc_tile_pool(name="qtp", bufs=1, side="left")
        QT = qtp.tile([P, KC, NQ], BF16, name="QT")
        caqp = tc.alloc_tile_pool(name="caqp", bufs=1, side="left")
        caQT = caqp.tile([P, KC, NQ], BF16, name="caQT")
        xbqp = tc.alloc_tile_pool(name="xbq", bufs=1, side="left")
        xTq_b = xbqp.tile([P, KC, NQ], BF16, name="xTq_b")
        otp = tc.alloc_tile_pool(name="otp", bufs=1, side="left")
        OT = otp.tile([P, KC, NQ], BF16, name="OT")

        # ---- right side, bottom of stack: weight ring + persistent inputs --
        ring = tc.alloc_tile_pool(name="ring", bufs=2, side="right")
        maskp = tc.alloc_tile_pool(name="maskp", bufs=1, side="right")
        mask_sb = maskp.tile([P, 4, NQ], BF16, name="mask_sb")

        # ---- phase 1: self-attn K^T / V projections (full seq) ----
        kvp = tc.alloc_tile_pool(name="kv", bufs=1, side="right")
        KT = kvp.tile([P, KC, S], BF16, name="KT")
        Vaug = kvp.tile([P, KC, 16, VW], BF16, name="Vaug")
        nc.vector.memset(Vaug[:, :, :, 64:65], 1.0)
        xmat = tc.alloc_tile_pool(name="xmat", bufs=1, side="right")
        xT_sb = xmat.tile([P, KC, T], BF16, name="xT_sb")

        # kick off input DMAs (order = DMA queue order). xT arrives in
        # column slices so the V projection (sc-outer) starts after the
        # first 256KB.
        wv = ring.tile([P, KC, E], BF16, name="w")
        nc.sync.dma_start(xT_sb[:, :, 0:128], xT[:, :, 0:128])
        nc.sync.dma_start(wv[:, 0:2, 0:512], wd["wv"][:, 0:2, 0:512])
        nc.sync.dma_start(xT_sb[:, :, 128:512], xT[:, :, 128:512])
        nc.sync.dma_start(wv[:, 2:8, 0:512], wd["wv"][:, 2:8, 0:512])
        nc.sync.dma_start(xT_sb[:, :, 512:1024], xT[:, :, 512:1024])
        nc.sync.dma_start(wv[:, :, 512:1024], wd["wv"][:, :, 512:1024])
        wk = ring.tile([P, KC, E], BF16, name="w")
        nc.sync.dma_start(wk[:, :, 0:512], wd["wk"][:, :, 0:512])
        nc.sync.dma_start(wk[:, :, 512:1024], wd["wk"][:, :, 512:1024])
        nc.sync.dma_start(xTq_b[:], xTq_bf)
        nc.sync.dma_start(xTq[:], xTq_f)

        # V: keys on partitions, per-head augmented slots. nt-outer: all
        # first-half-wv work (~27us of matmuls) runs before the second wv
        # half (which lands ~14us into the kernel) is needed.
        for nt in range(2):
            for sc2 in range(4):
                pair = psc.tile([P, 2, NQ], F32, name="ps")
                for sci in range(2):
                    sc = 2 * sc2 + sci
                    for kc in range(KC):
                        nc.tensor.matmul(pair[:, sci, :],
                                         xT_sb[:, kc, sc * P:(sc + 1) * P],
                                         wv[:, kc, nt * NQ:(nt + 1) * NQ],
                                         start=(kc == 0), stop=(kc == KC - 1))
                for sci in range(2):
                    sc = 2 * sc2 + sci
                    nc.scalar.activation(
                        Vaug[:, sc, nt * 8:(nt + 1) * 8, 0:64],
                        pair[:, sci, :].rearrange("p (h w) -> p h w", w=64),
                        AF.Copy)
        # K^T: kc-outer over 4-psum groups
        for nt in range(2):
            for g in range(2):
                pair = [psc.tile([P, 2, NQ], F32, name="ps") for _ in range(2)]
                for kc in range(KC):
                    for mi in range(4):
                        m = 4 * g + mi
                        nc.tensor.matmul(pair[mi // 2][:, mi % 2, :],
                                         wk[:, kc, m * P:(m + 1) * P],
                                         xT_sb[:, kc, nt * NQ:(nt + 1) * NQ],
                                         start=(kc == 0), stop=(kc == KC - 1))
                for mi in range(4):
                    m = 4 * g + mi
                    nc.scalar.activation(KT[:, m, nt * NQ:(nt + 1) * NQ],
                                         pair[mi // 2][:, mi % 2, :], AF.Copy)

        xmat.release()

        # ---- phase 2: Q^T projections (self + cross) ----
        wq = ring.tile([P, KC, E], BF16, name="w")
        nc.sync.dma_start(wq[:, :, 0:512], wd["wq"][:, :, 0:512])
        nc.sync.dma_start(wq[:, :, 512:1024], wd["wq"][:, :, 512:1024])
        cq = ring.tile([P, KC, E], BF16, name="w")
        nc.sync.dma_start(cq[:, :, 0:512], wd["cq"][:, :, 0:512])
        nc.sync.dma_start(cq[:, :, 512:1024], wd["cq"][:, :, 512:1024])
        nc.sync.dma_start(mask_sb[:], m01)

        for src, dst in ((wq, QT), (cq, caQT)):
            for mp in range(4):
                pair = psc.tile([P, 2, NQ], F32, name="ps")
                for mi in range(2):
                    m = 2 * mp + mi
                    for kc in range(KC):
                        nc.tensor.matmul(pair[:, mi, :],
                                         src[:, kc, m * P:(m + 1) * P],
                                         xTq_b[:, kc, :],
                                         start=(kc == 0), stop=(kc == KC - 1))
                for mi in range(2):
                    nc.scalar.activation(dst[:, 2 * mp + mi, :],
                                         pair[:, mi, :], AF.Copy)

        # ---- attention helper ----
        # Per group g: query cols [NG*g, NG*g+NG); key chunks c0..c0+nch.
        # masked groups apply the per-core 0/1 mask to their first mask_n
        # chunks (host guarantees all later chunks are unmasked).
        def attention(QTl, KTl, Vaugl, OTl, masked):
            expp = tc.alloc_tile_pool(name="expp", bufs=8, side="right")
            if masked:
                groups = ((0, 0, 8, True), (1, 4, 4, True))  # (g, c0, nch, msk)
            else:
                groups = ((0, 0, 8, False), (1, 0, 8, False))

            D = None
            for hp in range(8):   # head pairs: A=2hp (rows 0-63), B (64-127)
                if hp % 2 == 0:
                    # fresh denominator tile per 4-head batch; rows other
                    # than 0/32/64/96 stay 1.0 so the reciprocal + selector
                    # matmul read benign values.
                    D = bc.tile([P, NQ], F32, name="D")
                    nc.vector.memset(D[:], 1.0)
                exs = {0: [], 1: []}   # per h2: list of (g, ex_tile, s0, sn)
                for g, c0, nch, msk in groups:
                    q0 = NG * g
                    for o in range(0, nch, 4):
                        s0, sn = c0 + o, min(4, nch - o)
                        ps = [psc.tile([P, 4, NG], F32, name="ps")
                              for _ in range(2)]
                        for j in range(sn):
                            kc = s0 + j
                            for h2 in range(2):
                                lo = 64 * h2
                                nc.tensor.matmul(
                                    ps[h2][:, j, :],
                                    KTl[lo:lo + 64, hp, kc * P:(kc + 1) * P],
                                    QTl[lo:lo + 64, hp, q0:q0 + NG],
                                    start=True, stop=True)
                        for h2 in range(2):
                            ex = expp.tile([P, 4, NG], BF16, name="ex")
                            nc.scalar.activation(ex[:, 0:sn, :],
                                                 ps[h2][:, 0:sn, :],
                                                 AF.Exp, scale=EXP_SCALE)
                            if msk and o == 0:
                                nc.vector.tensor_tensor(
                                    ex[:, 0:sn, :], ex[:, 0:sn, :],
                                    mask_sb[:, 0:sn, q0:q0 + NG],
                                    AluOpType.mult)
                            exs[h2].append((g, ex, s0, sn))
                # AV per head: accumulate each group into its half of one
                # PSUM bank [65, 512]; row 64 = softmax denominator.
                for h2 in range(2):
                    h = 2 * hp + h2
                    lo = 64 * h2
                    pavt = pav.tile([P, NQ], F32, name="pav")
                    for g, c0, nch, msk in groups:
                        q0 = NG * g
                        tl = [t for t in exs[h2] if t[0] == g]
                        n_mm = sum(t[3] for t in tl)
                        i = 0
                        for _, ex, s0, sn in tl:
                            for j in range(sn):
                                nc.tensor.matmul(
                                    pavt[0:VW, q0:q0 + NG],
                                    Vaugl[:, s0 + j, h, :],
                                    ex[:, j, :],
                                    start=(i == 0), stop=(i == n_mm - 1))
                                i += 1
                    # values -> OT (DVE cast), denominator row -> D (scalar)
                    nc.vector.tensor_copy(OTl[lo:lo + 64, hp, :],
                                          pavt[0:64, :])
                    dp = 32 * (h % 4)
                    nc.scalar.activation(D[dp:dp + 1, :], pavt[64:65, :],
                                         AF.Copy)
                # after every 2 pairs (4 heads): reciprocal + apply
                if hp % 2 == 1:
                    Rf = bc.tile([P, NQ], F32, name="Rf")
                    nc.vector.reciprocal_approx_fast(Rf[:], D[:])
                    R8 = bc.tile([P, NQ], BF16, name="R8")
                    nc.vector.tensor_copy(R8[:], Rf[:])
                    for pi, hpx in enumerate((hp - 1, hp)):
                        Bp = pav.tile([P, NQ], F32, name="pav")
                        nc.tensor.matmul(Bp[:],
                                         sel_sb[:, 128 * pi:128 * pi + 128],
                                         R8[:], start=True, stop=True)
                        nc.vector.tensor_tensor(OTl[:, hpx, :],
                                                OTl[:, hpx, :],
                                                Bp[:], AluOpType.mult)
            expp.release()

        # ---- phase 3: self-attention ----
        attention(QT, KT, Vaug, OT, masked=True)
        kvp.release()
        maskp.release()

        # ---- phase 4a: self out-proj + residual into xTq ----
        emat = tc.alloc_tile_pool(name="emat", bufs=1, side="right")
        eT_sb = emat.tile([P, KC, S], BF16, name="eT_sb")
        nc.sync.dma_start(eT_sb[:], eT)
        wo = ring.tile([P, KC, E], BF16, name="w")
        nc.sync.dma_start(wo[:], wd["wo"])
        for mp in range(4):
            pair = psc.tile([P, 2, NQ], F32, name="ps")
            for mi in range(2):
                m = 2 * mp + mi
                for kc in range(KC):
                    nc.tensor.matmul(pair[:, mi, :],
                                     wo[:, kc, m * P:(m + 1) * P],
                                     OT[:, kc, :],
                                     start=(kc == 0), stop=(kc == KC - 1))
            for mi in range(2):
                m = 2 * mp + mi
                nc.vector.tensor_tensor(xTq[:, m, :], pair[:, mi, :],
                                        xTq[:, m, :], AluOpType.add)

        # ---- layernorm helper (in place over X [P,KC,NQ] f32) ----
        def layernorm(X, out_bf=None):
            sum_ps = pstat.tile([1, NQ], F32, name="s1")
            sq_ps = pstat.tile([1, NQ], F32, name="s2")
            for m in range(KC):
                xb = sqp.tile([P, NQ], BF16, name="xb")
                sq = sqp.tile([P, NQ], BF16, name="sq")
                nc.scalar.activation(xb[:], X[:, m, :], AF.Copy)
                nc.vector.tensor_tensor(sq[:], xb[:], xb[:], AluOpType.mult)
                nc.tensor.matmul(sum_ps[:], ones_bf[:], xb[:],
                                 start=(m == 0), stop=(m == KC - 1))
                nc.tensor.matmul(sq_ps[:], ones_bf[:], sq[:],
                                 start=(m == 0), stop=(m == KC - 1))
            mu = rows.tile([1, NQ], F32, name="mu")
            ex2 = rows.tile([1, NQ], F32, name="ex2")
            var = rows.tile([1, NQ], F32, name="var")
            std = rows.tile([1, NQ], F32, name="std")
            rstd = rows.tile([1, NQ], F32, name="rstd")
            nc.scalar.activation(mu[:], sum_ps[:], AF.Copy, scale=1.0 / E)
            nc.scalar.activation(ex2[:], sq_ps[:], AF.Copy, scale=1.0 / E)
            nc.vector.tensor_tensor(var[:], mu[:], mu[:], AluOpType.mult)
            nc.vector.tensor_tensor(var[:], ex2[:], var[:], AluOpType.subtract)
            # rstd = exp(-0.5*ln(var+eps)): keeps the exp table set resident
            # (sqrt lives in a different set -> ~2.7us table switch per LN)
            nc.scalar.activation(std[:], var[:], AF.Ln, bias=eps_t[0:1, :])
            nc.scalar.activation(rstd[:], std[:], AF.Exp, scale=-0.5)
            mub = rows.tile([1, NQ], BF16, name="mub")
            rsb = rows.tile([1, NQ], BF16, name="rsb")
            nc.scalar.activation(mub[:], mu[:], AF.Copy)
            nc.scalar.activation(rsb[:], rstd[:], AF.Copy)
            muB = pav.tile([P, NQ], F32, name="pav")
            nc.tensor.matmul(muB[:], ones_row[:], mub[:], start=True, stop=True)
            rsB = pav.tile([P, NQ], F32, name="pav")
            nc.tensor.matmul(rsB[:], ones_row[:], rsb[:], start=True, stop=True)
            for m in range(KC):
                nc.vector.tensor_tensor(X[:, m, :], X[:, m, :], muB[:],
                                        AluOpType.subtract)
                nc.vector.tensor_tensor(X[:, m, :], X[:, m, :], rsB[:],
                                        AluOpType.mult)
                if out_bf is not None:
                    nc.scalar.activation(out_bf[:, m, :], X[:, m, :], AF.Copy)

        def layernorm_bf(X, out_bf):
            sum_ps = pstat.tile([1, NQ], F32, name="s1")
            sq_ps = pstat.tile([1, NQ], F32, name="s2")
            for m in range(KC):
                xb = sqp.tile([P, NQ], BF16, name="xb")
                sq = sqp.tile([P, NQ], BF16, name="sq")
                nc.scalar.activation(xb[:], X[:, m, :], AF.Copy)
                nc.vector.tensor_tensor(sq[:], xb[:], xb[:], AluOpType.mult)
                nc.tensor.matmul(sum_ps[:], ones_bf[:], xb[:],
                                 start=(m == 0), stop=(m == KC - 1))
                nc.tensor.matmul(sq_ps[:], ones_bf[:], sq[:],
                                 start=(m == 0), stop=(m == KC - 1))
            mu = rows.tile([1, NQ], F32, name="mu")
            ex2 = rows.tile([1, NQ], F32, name="ex2")
            var = rows.tile([1, NQ], F32, name="var")
            std = rows.tile([1, NQ], F32, name="std")
            rstd = rows.tile([1, NQ], F32, name="rstd")
            nc.scalar.activation(mu[:], sum_ps[:], AF.Copy, scale=1.0 / E)
            nc.scalar.activation(ex2[:], sq_ps[:], AF.Copy, scale=1.0 / E)
            nc.vector.tensor_tensor(var[:], mu[:], mu[:], AluOpType.mult)
            nc.vector.tensor_tensor(var[:], ex2[:], var[:], AluOpType.subtract)
            # rstd = exp(-0.5*ln(var+eps)): keeps the exp table set resident
            # (sqrt lives in a different set -> ~2.7us table switch per LN)
            nc.scalar.activation(std[:], var[:], AF.Ln, bias=eps_t[0:1, :])
            nc.scalar.activation(rstd[:], std[:], AF.Exp, scale=-0.5)
            mub = rows.tile([1, NQ], BF16, name="mub")
            rsb = rows.tile([1, NQ], BF16, name="rsb")
            nc.scalar.activation(mub[:], mu[:], AF.Copy)
            nc.scalar.activation(rsb[:], rstd[:], AF.Copy)
            muB = pav.tile([P, NQ], F32, name="pav")
            nc.tensor.matmul(muB[:], ones_row[:], mub[:], start=True, stop=True)
            rsB = pav.tile([P, NQ], F32, name="pav")
            nc.tensor.matmul(rsB[:], ones_row[:], rsb[:], start=True, stop=True)
            rs_sb = bc.tile([P, NQ], BF16, name="rs_sb")
            nc.vector.tensor_copy(rs_sb[:], rsB[:])
            for m in range(KC):
                t = sqp.tile([P, NQ], BF16, name="t")
                nc.vector.tensor_tensor(t[:], X[:, m, :], muB[:],
                                        AluOpType.subtract)
                nc.vector.tensor_tensor(out_bf[:, m, :], t[:], rs_sb[:],
                                        AluOpType.mult)

        # ---- LN1 (xTq -> v^T); overlaps CA K/V projections ----
        layernorm(xTq)

        # ---- phase 5: cross-attn K^T / V projections from eT ----
        ck = ring.tile([P, KC, E], BF16, name="w")
        nc.sync.dma_start(ck[:], wd["ck"])
        cv = ring.tile([P, KC, E], BF16, name="w")
        nc.sync.dma_start(cv[:], wd["cv"])
        cakv = tc.alloc_tile_pool(name="cakv", bufs=1, side="right")
        caKT = cakv.tile([P, KC, S], BF16, name="caKT")
        caVaug = cakv.tile([P, KC, 16, VW], BF16, name="caVaug")
        nc.vector.memset(caVaug[:, :, :, 64:65], 1.0)

        for nt in range(2):
            for g in range(2):
                pair = [psc.tile([P, 2, NQ], F32, name="ps") for _ in range(2)]
                for kc in range(KC):
                    for mi in range(4):
                        m = 4 * g + mi
                        nc.tensor.matmul(pair[mi // 2][:, mi % 2, :],
                                         ck[:, kc, m * P:(m + 1) * P],
                                         eT_sb[:, kc, nt * NQ:(nt + 1) * NQ],
                                         start=(kc == 0), stop=(kc == KC - 1))
                for mi in range(4):
                    m = 4 * g + mi
                    nc.scalar.activation(caKT[:, m, nt * NQ:(nt + 1) * NQ],
                                         pair[mi // 2][:, mi % 2, :], AF.Copy)
        for sc in range(KC):
            pair = psc.tile([P, 2, NQ], F32, name="ps")
            for nt in range(2):
                for kc in range(KC):
                    nc.tensor.matmul(pair[:, nt, :],
                                     eT_sb[:, kc, sc * P:(sc + 1) * P],
                                     cv[:, kc, nt * NQ:(nt + 1) * NQ],
                                     start=(kc == 0), stop=(kc == KC - 1))
            for nt in range(2):
                nc.scalar.activation(
                    caVaug[:, sc, nt * 8:(nt + 1) * 8, 0:64],
                    pair[:, nt, :].rearrange("p (h w) -> p h w", w=64), AF.Copy)

        # ---- phase 6: cross-attention ----
        attention(caQT, caKT, caVaug, OT, masked=False)
        cakv.release()
        emat.release()

        # ---- phase 7: cross out-proj + residual (v^T in xTq) + LN2 ----
        co = ring.tile([P, KC, E], BF16, name="w")
        nc.sync.dma_start(co[:], wd["co"])
        btfp = tc.alloc_tile_pool(name="btf", bufs=1, side="left")
        bT_bf = btfp.tile([P, KC, NQ], BF16, name="bT_bf")
        for mp in range(4):
            pair = psc.tile([P, 2, NQ], F32, name="ps")
            for mi in range(2):
                m = 2 * mp + mi
                for kc in range(KC):
                    nc.tensor.matmul(pair[:, mi, :],
                                     co[:, kc, m * P:(m + 1) * P],
                                     OT[:, kc, :],
                                     start=(kc == 0), stop=(kc == KC - 1))
            for mi in range(2):
                m = 2 * mp + mi
                nc.vector.tensor_tensor(xTq[:, m, :], pair[:, mi, :],
                                        xTq[:, m, :], AluOpType.add)
        layernorm_bf(xTq, bT_bf)   # b^T in bf16; xTq stays un-normalized

        # ---- phase 8: FFN up (relu on DVE); prefetch FFN-down weights ----
        htp = tc.alloc_tile_pool(name="ht", bufs=1, side="right")
        HT = htp.tile([P, KH, NQ], BF16, name="HT")
        wp2 = tc.alloc_tile_pool(name="wp2", bufs=2, side="right")
        for g in range(4):
            w1g = ring.tile([P, KC, 1024], BF16, name="w")
            nc.sync.dma_start(w1g[:, :, 0:512], w1[g][:, :, 0:512])
            nc.sync.dma_start(w1g[:, :, 512:1024], w1[g][:, :, 512:1024])
            for half in range(2):
                pair = [psc.tile([P, 2, NQ], F32, name="ps") for _ in range(2)]
                for kc in range(KC):
                    for mi in range(4):
                        mh = 4 * half + mi
                        nc.tensor.matmul(pair[mi // 2][:, mi % 2, :],
                                         w1g[:, kc, mh * P:(mh + 1) * P],
                                         bT_bf[:, kc, :],
                                         start=(kc == 0), stop=(kc == KC - 1))
                for mi in range(4):
                    mh = 4 * half + mi
                    nc.vector.tensor_scalar_max(HT[:, g * 8 + mh, :],
                                                pair[mi // 2][:, mi % 2, :],
                                                0.0)

        # ---- phase 9: FFN down + residual + LN3 + store ----
        outp = tc.alloc_tile_pool(name="outp", bufs=1, side="left")
        outT_sb = outp.tile([P, KC, NQ], F32, name="outT_sb")
        sum_ps = pstat.tile([1, NQ], F32, name="s1")
        sq_ps = pstat.tile([1, NQ], F32, name="s2")
        for mp in range(4):
            w2g = wp2.tile([P, KH, 256], BF16, name="w2g")
            nc.sync.dma_start(w2g[:], w2[mp])
            pair = psc.tile([P, 2, NQ], F32, name="ps")
            for mi in range(2):
                m = 2 * mp + mi
                for kc2 in range(KH):
                    nc.tensor.matmul(pair[:, mi, :],
                                     w2g[:, kc2, mi * P:(mi + 1) * P],
                                     HT[:, kc2, :],
                                     start=(kc2 == 0), stop=(kc2 == KH - 1))
            for mi in range(2):
                m = 2 * mp + mi
                nc.vector.tensor_tensor(outT_sb[:, m, :], pair[:, mi, :],
                                        bT_bf[:, m, :], AluOpType.add)
                # incremental LN3 stats
                xb = sqp.tile([P, NQ], BF16, name="xb")
                sq = sqp.tile([P, NQ], BF16, name="sq")
                nc.scalar.activation(xb[:], outT_sb[:, m, :], AF.Copy)
                nc.vector.tensor_tensor(sq[:], xb[:], xb[:], AluOpType.mult)
                nc.tensor.matmul(sum_ps[:], ones_bf[:], xb[:],
                                 start=(m == 0), stop=(m == KC - 1))
                nc.tensor.matmul(sq_ps[:], ones_bf[:], sq[:],
                                 start=(m == 0), stop=(m == KC - 1))
        mu = rows.tile([1, NQ], F32, name="mu")
        ex2 = rows.tile([1, NQ], F32, name="ex2")
        var = rows.tile([1, NQ], F32, name="var")
        std = rows.tile([1, NQ], F32, name="std")
        rstd = rows.tile([1, NQ], F32, name="rstd")
        nc.scalar.activation(mu[:], sum_ps[:], AF.Copy, scale=1.0 / E)
        nc.scalar.activation(ex2[:], sq_ps[:], AF.Copy, scale=1.0 / E)
        nc.vector.tensor_tensor(var[:], mu[:], mu[:], AluOpType.mult)
        nc.vector.tensor_tensor(var[:], ex2[:], var[:], AluOpType.subtract)
        nc.scalar.activation(std[:], var[:], AF.Ln, bias=eps_t[0:1, :])
        nc.scalar.activation(rstd[:], std[:], AF.Exp, scale=-0.5)
        mub = rows.tile([1, NQ], BF16, name="mub")
        rsb = rows.tile([1, NQ], BF16, name="rsb")
        nc.scalar.activation(mub[:], mu[:], AF.Copy)
        nc.scalar.activation(rsb[:], rstd[:], AF.Copy)
        muB = pav.tile([P, NQ], F32, name="pav")
        nc.tensor.matmul(muB[:], ones_row[:], mub[:], start=True, stop=True)
        rsB = pav.tile([P, NQ], F32, name="pav")
        nc.tensor.matmul(rsB[:], ones_row[:], rsb[:], start=True, stop=True)
        for m in range(KC):
            nc.vector.tensor_tensor(outT_sb[:, m, :], outT_sb[:, m, :],
                                    muB[:], AluOpType.subtract)
            nc.vector.tensor_tensor(outT_sb[:, m, :], outT_sb[:, m, :],
                                    rsB[:], AluOpType.mult)
            nc.sync.dma_start(outT[:, m, :], outT_sb[:, m, :])

        # right side LIFO
        wp2.release()
        htp.release()
        ring.release()
        # left side LIFO
        outp.release()
        btfp.release()
        otp.release()
        xbqp.release()
        caqp.release()
        qtp.release()
        xtqp.release()
        sqp.release()
        bc.release()
        rows.release()
        pstat.release()
        pav.release()
        psc.release()
        const.release()

    nc.compile()
    return nc


def get_nc():
    global _NC_CACHE
    if _NC_CACHE is None:
        _NC_CACHE = build_nc()
    return _NC_CACHE


def _tile_pc(w):
    """[K*P, N] -> [P, K, N] contiguous chunk-tiled copy."""
    kp, n = w.shape
    return np.ascontiguousarray(
        w.reshape(kp // P, P, n).transpose(1, 0, 2))


def _qblocks(parity):
    return (0, 1, 6, 7) if parity == 0 else (2, 3, 4, 5)


def _qidx(parity):
    return np.concatenate([np.arange(128 * b, 128 * b + 128)
                           for b in _qblocks(parity)])


def make_in_maps(inputs):
    """Build per-core input maps from the full (unsharded) input dict."""
    bf = ml_dtypes.bfloat16
    tgt = np.asarray(inputs['target'], dtype=np.float32)
    enc = np.asarray(inputs['enc_src'], dtype=np.float32)
    wcast = {}
    for src_name, name in (('sa_wq', 'wq'), ('sa_wk', 'wk'), ('sa_wv', 'wv'),
                           ('sa_wo', 'wo'), ('ca_wq', 'cq'), ('ca_wk', 'ck'),
                           ('ca_wv', 'cv'), ('ca_wo', 'co')):
        wcast[name] = _tile_pc(np.asarray(inputs[src_name], np.float32)
                               .astype(bf))
    w1f = np.asarray(inputs['ff_w1'], np.float32).astype(bf)
    wcast['w1'] = np.ascontiguousarray(
        w1f.reshape(KC, P, 4, 1024).transpose(2, 1, 0, 3))
    w2f = np.asarray(inputs['ff_w2'], np.float32).astype(bf)
    wcast['w2'] = np.ascontiguousarray(
        w2f.reshape(KH, P, 4, 256).transpose(2, 1, 0, 3))
    sel = np.zeros((P, 256), np.float32)
    for r in range(4):
        sel[32 * r, 128 * (r // 2) + 64 * (r % 2):
            128 * (r // 2) + 64 * (r % 2) + 64] = 1.0
    wcast['sel4'] = np.ascontiguousarray(sel.astype(bf))
    in_maps = []
    k_loc = np.arange(P, dtype=np.int64)
    for c in range(8):
        b, par = c // 2, c % 2
        qi = _qidx(par)
        # mask: [:, ch, 0:256] covers G0 chunks 0-3 over queries qi[0:256];
        #       [:, ch, 256:512] covers G1 chunks 4-7 over queries qi[256:].
        m = np.zeros((P, 4, NQ), dtype=np.float32)
        for ch in range(4):
            kg0 = 128 * ch + k_loc          # G0 chunk = ch
            kg1 = 128 * (4 + ch) + k_loc    # G1 chunk = 4 + ch
            m[:, ch, 0:NG] = (kg0[:, None] >= qi[None, 0:NG])
            m[:, ch, NG:] = (kg1[:, None] >= qi[None, NG:])
        xTb = np.ascontiguousarray(tgt[b].T)
        xTbq = np.ascontiguousarray(xTb[:, qi])
        mm = {
            'xT': _tile_pc(xTb.astype(bf)),
            'xTq_bf': _tile_pc(xTbq.astype(bf)),
            'xTq_f': _tile_pc(xTbq),
            'eT': _tile_pc(enc[b].T.astype(bf)),
            'm01': np.ascontiguousarray(m.astype(bf)),
        }
        mm.update(wcast)
        in_maps.append(mm)
    return in_maps


def assemble(results):
    out = np.empty((4, T, E), dtype=np.float32)
    for c in range(8):
        b, par = c // 2, c % 2
        qi = _qidx(par)
        o = results[c]['outT']          # [P, KC, NQ]
        out[b, qi, :] = o.transpose(2, 1, 0).reshape(NQ, E)
    return out


def kernel(**inputs):
    nc = get_nc()
    in_maps = make_in_maps(inputs)
    res = run_bass_kernel_spmd(nc, in_maps, core_ids=list(range(8)))
    return assemble(res.results)


# revision 26
# speedup vs baseline: 1.0554x; 1.0288x over previous
"""Trainium2 Bass kernel for nn_DecoderBlock (B=4, T=S=1024, E=1024, H=16).

v5: startup, LN2-bridge, and tail improvements on the v3 attention
restructure.

Sharding: 8-way, zero-collective. Core c handles batch b=c//2; parity
p=c%2 selects the query-block set (128-row blocks of the sequence):
p=0 -> blocks {0,1,6,7}, p=1 -> blocks {2,3,4,5}. Queries are stored
grouped as [G0: 256 | G1: 256] columns. This balances the anti-causal
mask (query q attends keys k >= q) so every core runs the identical
instruction stream (SPMD) while skipping fully-masked score blocks:
G0 processes key chunks 0..7, G1 only 4..7 (12 chunk-blocks per head
instead of 16); per-core 0/1 masks (inputs) cover the partially-masked
chunks.

Other points:
  - xT arrives in column slices and the V projection runs first, so the
    first matmul starts ~2us in.
  - LN2 emits bf16 bT_bf directly (one sub + one mult per chunk); the
    FFN residual reads bT_bf, halving the LN2->FFN bridge.
  - FFN-down weights prefetch into their own pool during FFN-up.
  - AV accumulates per group into one PSUM bank [65, 512] per head
    (ones column -> denominator at row 64); softmax normalization is
    a batched reciprocal every 4 heads + one selector matmul and one
    [128,512] DVE multiply per head pair.
  - FFN relu on DVE; FFN-down weights prefetched into their own pool.
  - Weight DMAs are sliced so first consumers start early.
"""
import numpy as np

import ml_dtypes
import concourse.bacc as bacc
import concourse.mybir as mybir
import concourse.tile as tile
from concourse.alu_op_type import AluOpType
from concourse.bass_utils import run_bass_kernel_spmd

P = 128
E = 1024
T = 1024
S = 1024
NQ = 512          # query rows per core
NG = 256          # query rows per group
H4 = 4096         # FFN hidden
KC = 8            # E / P chunks
KH = 32           # H4 / P chunks
VW = 65           # V columns per head incl. ones column
F32 = mybir.dt.float32
BF16 = mybir.dt.bfloat16
AF = mybir.ActivationFunctionType
EXP_SCALE = 1.0 / 8.0   # 1/sqrt(head_size)
EPS = 1e-5

_NC_CACHE = None


def build_nc():
    nc = bacc.Bacc("TRN2", target_bir_lowering=False, debug=False)

    def din(n, s, dt=BF16):
        return nc.dram_tensor(n, s, dt, kind="ExternalInput").ap()

    xT = din("xT", [P, KC, T])               # target^T, pre-tiled
    xTq_bf = din("xTq_bf", [P, KC, NQ])      # query slice bf16 (grouped)
    xTq_f = din("xTq_f", [P, KC, NQ], F32)   # query slice f32 (residual)
    eT = din("eT", [P, KC, S])               # enc_src^T
    m01 = din("m01", [P, 4, NQ])             # keep-mask: [:, :, 0:256] = G0
    #                                          chunks 0-3, [:, :, 256:] = G1
    #                                          chunks 4-7
    sel4 = din("sel4", [P, 256])             # denominator broadcast selectors
    wd = {n: din(n, [P, KC, E]) for n in
          ("wq", "wk", "wv", "wo", "cq", "ck", "cv", "co")}
    w1 = din("w1", [4, P, KC, 1024])         # FFN up, col-groups of 1024
    w2 = din("w2", [4, P, KH, 256])          # FFN down, col-groups of 256
    outT = nc.dram_tensor("outT", [P, KC, NQ], F32, kind="ExternalOutput").ap()

    with tile.TileContext(nc) as tc:
        # ---- constants ----
        const = tc.alloc_tile_pool(name="const", bufs=1)
        ones_f = const.tile([P, 1], F32)
        nc.vector.memset(ones_f[:], 1.0)
        ones_bf = const.tile([P, 1], BF16)
        nc.scalar.activation(ones_bf[:], ones_f[:], AF.Copy)
        eps_t = const.tile([P, 1], F32)
        nc.vector.memset(eps_t[:], EPS)
        ones_row = const.tile([1, P], BF16)
        nc.vector.memset(ones_row[:], 1.0)
        # denominator broadcast selectors (DMA'd, not memset — avoids
        # scheduler-ordering hazards). cols 0:128 = even-pair pattern
        # (partition 0 -> out 0-63, partition 32 -> out 64-127); cols
        # 128:256 = odd-pair pattern (64 -> 0-63, 96 -> 64-127).
        sel_sb = const.tile([P, 256], BF16)
        nc.sync.dma_start(sel_sb[:], sel4)

        # ---- PSUM pools (exactly 8 banks: 4 + 2 + 2) ----
        psc = tc.alloc_tile_pool(name="psc", bufs=2, space="PSUM")   # 2-bank tiles
        pav = tc.alloc_tile_pool(name="pav", bufs=2, space="PSUM")   # [P,NQ]
        pstat = tc.alloc_tile_pool(name="pstat", bufs=1, space="PSUM")  # 2x [1,NQ]

        # ---- small SBUF pools ----
        rows = tc.alloc_tile_pool(name="rows", bufs=1)   # [8,NQ]/[1,NQ] stats
        bc = tc.alloc_tile_pool(name="bc", bufs=2)       # [P,NQ] recip/broadcast
        sqp = tc.alloc_tile_pool(name="sqp", bufs=2)     # LN bf16 shadows

        # ---- left side: long-lived activations ----
        xtqp = tc.alloc_tile_pool(name="xtqp", bufs=1, side="left")
        xTq = xtqp.tile([P, KC, NQ], F32, name="xTq")    # residual stream
        qtp = tc.alloc_tile_pool(name="qtp", bufs=1, side="left")
        QT = qtp.tile([P, KC, NQ], BF16, name="QT")
        caqp = tc.alloc_tile_pool(name="caqp", bufs=1, side="left")
        caQT = caqp.tile([P, KC, NQ], BF16, name="caQT")
        xbqp = tc.alloc_tile_pool(name="xbq", bufs=1, side="left")
        xTq_b = xbqp.tile([P, KC, NQ], BF16, name="xTq_b")
        otp = tc.alloc_tile_pool(name="otp", bufs=1, side="left")
        OT = otp.tile([P, KC, NQ], BF16, name="OT")

        # ---- right side, bottom of stack: weight ring + persistent inputs --
        ring = tc.alloc_tile_pool(name="ring", bufs=2, side="right")
        maskp = tc.alloc_tile_pool(name="maskp", bufs=1, side="right")
        mask_sb = maskp.tile([P, 4, NQ], BF16, name="mask_sb")

        # ---- phase 1: self-attn K^T / V projections (full seq) ----
        kvp = tc.alloc_tile_pool(name="kv", bufs=1, side="right")
        KT = kvp.tile([P, KC, S], BF16, name="KT")
        Vaug = kvp.tile([P, KC, 16, VW], BF16, name="Vaug")
        nc.vector.memset(Vaug[:, :, :, 64:65], 1.0)
        xmat = tc.alloc_tile_pool(name="xmat", bufs=1, side="right")
        xT_sb = xmat.tile([P, KC, T], BF16, name="xT_sb")

        # kick off input DMAs (order = DMA queue order). xT arrives in
        # column slices so the V projection (sc-outer) starts after the
        # first 256KB.
        wv = ring.tile([P, KC, E], BF16, name="w")
        nc.sync.dma_start(xT_sb[:, :, 0:128], xT[:, :, 0:128])
        nc.sync.dma_start(wv[:, 0:2, 0:512], wd["wv"][:, 0:2, 0:512])
        nc.sync.dma_start(xT_sb[:, :, 128:512], xT[:, :, 128:512])
        nc.sync.dma_start(wv[:, 2:8, 0:512], wd["wv"][:, 2:8, 0:512])
        nc.sync.dma_start(xT_sb[:, :, 512:1024], xT[:, :, 512:1024])
        nc.sync.dma_start(wv[:, :, 512:1024], wd["wv"][:, :, 512:1024])
        wk = ring.tile([P, KC, E], BF16, name="w")
        nc.sync.dma_start(wk[:, :, 0:512], wd["wk"][:, :, 0:512])
        nc.sync.dma_start(wk[:, :, 512:1024], wd["wk"][:, :, 512:1024])
        nc.sync.dma_start(xTq_b[:], xTq_bf)
        nc.sync.dma_start(xTq[:], xTq_f)

        # V: keys on partitions, per-head augmented slots. nt-outer: all
        # first-half-wv work (~27us of matmuls) runs before the second wv
        # half (which lands ~14us into the kernel) is needed.
        for nt in range(2):
            for sc2 in range(4):
                pair = psc.tile([P, 2, NQ], F32, name="ps")
                for sci in range(2):
                    sc = 2 * sc2 + sci
                    for kc in range(KC):
                        nc.tensor.matmul(pair[:, sci, :],
                                         xT_sb[:, kc, sc * P:(sc + 1) * P],
                                         wv[:, kc, nt * NQ:(nt + 1) * NQ],
                                         start=(kc == 0), stop=(kc == KC - 1))
                for sci in range(2):
                    sc = 2 * sc2 + sci
                    nc.scalar.activation(
                        Vaug[:, sc, nt * 8:(nt + 1) * 8, 0:64],
                        pair[:, sci, :].rearrange("p (h w) -> p h w", w=64),
                        AF.Copy)
        # K^T: kc-outer over 4-psum groups
        for nt in range(2):
            for g in range(2):
                pair = [psc.tile([P, 2, NQ], F32, name="ps") for _ in range(2)]
                for kc in range(KC):
                    for mi in range(4):
                        m = 4 * g + mi
                        nc.tensor.matmul(pair[mi // 2][:, mi % 2, :],
                                         wk[:, kc, m * P:(m + 1) * P],
                                         xT_sb[:, kc, nt * NQ:(nt + 1) * NQ],
                                         start=(kc == 0), stop=(kc == KC - 1))
                for mi in range(4):
                    m = 4 * g + mi
                    nc.scalar.activation(KT[:, m, nt * NQ:(nt + 1) * NQ],
                                         pair[mi // 2][:, mi % 2, :], AF.Copy)

        xmat.release()

        # ---- phase 2: Q^T projections (self + cross) ----
        wq = ring.tile([P, KC, E], BF16, name="w")
        nc.sync.dma_start(wq[:, :, 0:512], wd["wq"][:, :, 0:512])
        nc.sync.dma_start(wq[:, :, 512:1024], wd["wq"][:, :, 512:1024])
        cq = ring.tile([P, KC, E], BF16, name="w")
        nc.sync.dma_start(cq[:, :, 0:512], wd["cq"][:, :, 0:512])
        nc.sync.dma_start(cq[:, :, 512:1024], wd["cq"][:, :, 512:1024])
        nc.sync.dma_start(mask_sb[:], m01)

        for src, dst in ((wq, QT), (cq, caQT)):
            for mp in range(4):
                pair = psc.tile([P, 2, NQ], F32, name="ps")
                for mi in range(2):
                    m = 2 * mp + mi
                    for kc in range(KC):
                        nc.tensor.matmul(pair[:, mi, :],
                                         src[:, kc, m * P:(m + 1) * P],
                                         xTq_b[:, kc, :],
                                         start=(kc == 0), stop=(kc == KC - 1))
                for mi in range(2):
                    nc.scalar.activation(dst[:, 2 * mp + mi, :],
                                         pair[:, mi, :], AF.Copy)

        # ---- attention helper ----
        # Per group g: query cols [NG*g, NG*g+NG); key chunks c0..c0+nch.
        # masked groups apply the per-core 0/1 mask to their first mask_n
        # chunks (host guarantees all later chunks are unmasked).
        def attention(QTl, KTl, Vaugl, OTl, masked):
            expp = tc.alloc_tile_pool(name="expp", bufs=8, side="right")
            if masked:
                groups = ((0, 0, 8, True), (1, 4, 4, True))  # (g, c0, nch, msk)
            else:
                groups = ((0, 0, 8, False), (1, 0, 8, False))

            D = None
            for hp in range(8):   # head pairs: A=2hp (rows 0-63), B (64-127)
                if hp % 2 == 0:
                    # fresh denominator tile per 4-head batch; rows other
                    # than 0/32/64/96 stay 1.0 so the reciprocal + selector
                    # matmul read benign values.
                    D = bc.tile([P, NQ], F32, name="D")
                    nc.vector.memset(D[:], 1.0)
                exs = {0: [], 1: []}   # per h2: list of (g, ex_tile, s0, sn)
                for g, c0, nch, msk in groups:
                    q0 = NG * g
                    for o in range(0, nch, 4):
                        s0, sn = c0 + o, min(4, nch - o)
                        ps = [psc.tile([P, 4, NG], F32, name="ps")
                              for _ in range(2)]
                        for j in range(sn):
                            kc = s0 + j
                            for h2 in range(2):
                                lo = 64 * h2
                                nc.tensor.matmul(
                                    ps[h2][:, j, :],
                                    KTl[lo:lo + 64, hp, kc * P:(kc + 1) * P],
                                    QTl[lo:lo + 64, hp, q0:q0 + NG],
                                    start=True, stop=True)
                        for h2 in range(2):
                            ex = expp.tile([P, 4, NG], BF16, name="ex")
                            nc.scalar.activation(ex[:, 0:sn, :],
                                                 ps[h2][:, 0:sn, :],
                                                 AF.Exp, scale=EXP_SCALE)
                            if msk and o == 0:
                                nc.vector.tensor_tensor(
                                    ex[:, 0:sn, :], ex[:, 0:sn, :],
                                    mask_sb[:, 0:sn, q0:q0 + NG],
                                    AluOpType.mult)
                            exs[h2].append((g, ex, s0, sn))
                # AV per head: accumulate each group into its half of one
                # PSUM bank [65, 512]; row 64 = softmax denominator.
                for h2 in range(2):
                    h = 2 * hp + h2
                    lo = 64 * h2
                    pavt = pav.tile([P, NQ], F32, name="pav")
                    for g, c0, nch, msk in groups:
                        q0 = NG * g
                        tl = [t for t in exs[h2] if t[0] == g]
                        n_mm = sum(t[3] for t in tl)
                        i = 0
                        for _, ex, s0, sn in tl:
                            for j in range(sn):
                                nc.tensor.matmul(
                                    pavt[0:VW, q0:q0 + NG],
                                    Vaugl[:, s0 + j, h, :],
                                    ex[:, j, :],
                                    start=(i == 0), stop=(i == n_mm - 1))
                                i += 1
                    # values -> OT (DVE cast), denominator row -> D (scalar)
                    nc.vector.tensor_copy(OTl[lo:lo + 64, hp, :],
                                          pavt[0:64, :])
                    dp = 32 * (h % 4)
                    # on DVE: the scalar engine is exp-bound in attention
                    # windows, and the reciprocal consumer is DVE anyway
                    nc.vector.tensor_copy(D[dp:dp + 1, :], pavt[64:65, :])
                # after every 2 pairs (4 heads): reciprocal + apply
                if hp % 2 == 1:
                    Rf = bc.tile([P, NQ], F32, name="Rf")
                    nc.vector.reciprocal_approx_fast(Rf[:], D[:])
                    R8 = bc.tile([P, NQ], BF16, name="R8")
                    nc.vector.tensor_copy(R8[:], Rf[:])
                    for pi, hpx in enumerate((hp - 1, hp)):
                        Bp = pav.tile([P, NQ], F32, name="pav")
                        nc.tensor.matmul(Bp[:],
                                         sel_sb[:, 128 * pi:128 * pi + 128],
                                         R8[:], start=True, stop=True)
                        nc.vector.tensor_tensor(OTl[:, hpx, :],
                                                OTl[:, hpx, :],
                                                Bp[:], AluOpType.mult)
            expp.release()

        # ---- phase 3: self-attention ----
        attention(QT, KT, Vaug, OT, masked=True)
        kvp.release()
        maskp.release()

        # ---- phase 4a: self out-proj + residual into xTq ----
        emat = tc.alloc_tile_pool(name="emat", bufs=1, side="right")
        eT_sb = emat.tile([P, KC, S], BF16, name="eT_sb")
        nc.sync.dma_start(eT_sb[:], eT)
        wo = ring.tile([P, KC, E], BF16, name="w")
        nc.sync.dma_start(wo[:], wd["wo"])
        for mp in range(4):
            pair = psc.tile([P, 2, NQ], F32, name="ps")
            for mi in range(2):
                m = 2 * mp + mi
                for kc in range(KC):
                    nc.tensor.matmul(pair[:, mi, :],
                                     wo[:, kc, m * P:(m + 1) * P],
                                     OT[:, kc, :],
                                     start=(kc == 0), stop=(kc == KC - 1))
            for mi in range(2):
                m = 2 * mp + mi
                nc.vector.tensor_tensor(xTq[:, m, :], pair[:, mi, :],
                                        xTq[:, m, :], AluOpType.add)

        # ---- layernorm helper (in place over X [P,KC,NQ] f32) ----
        def layernorm(X, out_bf=None):
            sum_ps = pstat.tile([1, NQ], F32, name="s1")
            sq_ps = pstat.tile([1, NQ], F32, name="s2")
            for m in range(KC):
                xb = sqp.tile([P, NQ], BF16, name="xb")
                sq = sqp.tile([P, NQ], BF16, name="sq")
                nc.scalar.activation(xb[:], X[:, m, :], AF.Copy)
                nc.vector.tensor_tensor(sq[:], xb[:], xb[:], AluOpType.mult)
                nc.tensor.matmul(sum_ps[:], ones_bf[:], xb[:],
                                 start=(m == 0), stop=(m == KC - 1))
                nc.tensor.matmul(sq_ps[:], ones_bf[:], sq[:],
                                 start=(m == 0), stop=(m == KC - 1))
            mu = rows.tile([1, NQ], F32, name="mu")
            ex2 = rows.tile([1, NQ], F32, name="ex2")
            var = rows.tile([1, NQ], F32, name="var")
            std = rows.tile([1, NQ], F32, name="std")
            rstd = rows.tile([1, NQ], F32, name="rstd")
            nc.scalar.activation(mu[:], sum_ps[:], AF.Copy, scale=1.0 / E)
            nc.scalar.activation(ex2[:], sq_ps[:], AF.Copy, scale=1.0 / E)
            nc.vector.tensor_tensor(var[:], mu[:], mu[:], AluOpType.mult)
            nc.vector.tensor_tensor(var[:], ex2[:], var[:], AluOpType.subtract)
            # rstd = exp(-0.5*ln(var+eps)): keeps the exp table set resident
            # (sqrt lives in a different set -> ~2.7us table switch per LN)
            nc.scalar.activation(std[:], var[:], AF.Ln, bias=eps_t[0:1, :])
            nc.scalar.activation(rstd[:], std[:], AF.Exp, scale=-0.5)
            mub = rows.tile([1, NQ], BF16, name="mub")
            rsb = rows.tile([1, NQ], BF16, name="rsb")
            nc.scalar.activation(mub[:], mu[:], AF.Copy)
            nc.scalar.activation(rsb[:], rstd[:], AF.Copy)
            muB = pav.tile([P, NQ], F32, name="pav")
            nc.tensor.matmul(muB[:], ones_row[:], mub[:], start=True, stop=True)
            rsB = pav.tile([P, NQ], F32, name="pav")
            nc.tensor.matmul(rsB[:], ones_row[:], rsb[:], start=True, stop=True)
            for m in range(KC):
                nc.vector.tensor_tensor(X[:, m, :], X[:, m, :], muB[:],
                                        AluOpType.subtract)
                nc.vector.tensor_tensor(X[:, m, :], X[:, m, :], rsB[:],
                                        AluOpType.mult)
                if out_bf is not None:
                    nc.scalar.activation(out_bf[:, m, :], X[:, m, :], AF.Copy)

        def layernorm_bf(X, out_bf):
            sum_ps = pstat.tile([1, NQ], F32, name="s1")
            sq_ps = pstat.tile([1, NQ], F32, name="s2")
            for m in range(KC):
                xb = sqp.tile([P, NQ], BF16, name="xb")
                sq = sqp.tile([P, NQ], BF16, name="sq")
                nc.scalar.activation(xb[:], X[:, m, :], AF.Copy)
                nc.vector.tensor_tensor(sq[:], xb[:], xb[:], AluOpType.mult)
                nc.tensor.matmul(sum_ps[:], ones_bf[:], xb[:],
                                 start=(m == 0), stop=(m == KC - 1))
                nc.tensor.matmul(sq_ps[:], ones_bf[:], sq[:],
                                 start=(m == 0), stop=(m == KC - 1))
            mu = rows.tile([1, NQ], F32, name="mu")
            ex2 = rows.tile([1, NQ], F32, name="ex2")
            var = rows.tile([1, NQ], F32, name="var")
            std = rows.tile([1, NQ], F32, name="std")
            rstd = rows.tile([1, NQ], F32, name="rstd")
            nc.scalar.activation(mu[:], sum_ps[:], AF.Copy, scale=1.0 / E)
            nc.scalar.activation(ex2[:], sq_ps[:], AF.Copy, scale=1.0 / E)
            nc.vector.tensor_tensor(var[:], mu[:], mu[:], AluOpType.mult)
            nc.vector.tensor_tensor(var[:], ex2[:], var[:], AluOpType.subtract)
            # rstd = exp(-0.5*ln(var+eps)): keeps the exp table set resident
            # (sqrt lives in a different set -> ~2.7us table switch per LN)
            nc.scalar.activation(std[:], var[:], AF.Ln, bias=eps_t[0:1, :])
            nc.scalar.activation(rstd[:], std[:], AF.Exp, scale=-0.5)
            mub = rows.tile([1, NQ], BF16, name="mub")
            rsb = rows.tile([1, NQ], BF16, name="rsb")
            nc.scalar.activation(mub[:], mu[:], AF.Copy)
            nc.scalar.activation(rsb[:], rstd[:], AF.Copy)
            muB = pav.tile([P, NQ], F32, name="pav")
            nc.tensor.matmul(muB[:], ones_row[:], mub[:], start=True, stop=True)
            rsB = pav.tile([P, NQ], F32, name="pav")
            nc.tensor.matmul(rsB[:], ones_row[:], rsb[:], start=True, stop=True)
            rs_sb = bc.tile([P, NQ], BF16, name="rs_sb")
            nc.vector.tensor_copy(rs_sb[:], rsB[:])
            for m in range(KC):
                t = sqp.tile([P, NQ], BF16, name="t")
                nc.vector.tensor_tensor(t[:], X[:, m, :], muB[:],
                                        AluOpType.subtract)
                nc.vector.tensor_tensor(out_bf[:, m, :], t[:], rs_sb[:],
                                        AluOpType.mult)

        # ---- LN1 (xTq -> v^T); overlaps CA K/V projections ----
        layernorm(xTq)

        # ---- phase 5: cross-attn K^T / V projections from eT ----
        ck = ring.tile([P, KC, E], BF16, name="w")
        nc.sync.dma_start(ck[:], wd["ck"])
        cv = ring.tile([P, KC, E], BF16, name="w")
        nc.sync.dma_start(cv[:], wd["cv"])
        cakv = tc.alloc_tile_pool(name="cakv", bufs=1, side="right")
        caKT = cakv.tile([P, KC, S], BF16, name="caKT")
        caVaug = cakv.tile([P, KC, 16, VW], BF16, name="caVaug")
        nc.vector.memset(caVaug[:, :, :, 64:65], 1.0)

        for nt in range(2):
            for g in range(2):
                pair = [psc.tile([P, 2, NQ], F32, name="ps") for _ in range(2)]
                for kc in range(KC):
                    for mi in range(4):
                        m = 4 * g + mi
                        nc.tensor.matmul(pair[mi // 2][:, mi % 2, :],
                                         ck[:, kc, m * P:(m + 1) * P],
                                         eT_sb[:, kc, nt * NQ:(nt + 1) * NQ],
                                         start=(kc == 0), stop=(kc == KC - 1))
                for mi in range(4):
                    m = 4 * g + mi
                    nc.scalar.activation(caKT[:, m, nt * NQ:(nt + 1) * NQ],
                                         pair[mi // 2][:, mi % 2, :], AF.Copy)
        for sc in range(KC):
            pair = psc.tile([P, 2, NQ], F32, name="ps")
            for nt in range(2):
                for kc in range(KC):
                    nc.tensor.matmul(pair[:, nt, :],
                                     eT_sb[:, kc, sc * P:(sc + 1) * P],
                                     cv[:, kc, nt * NQ:(nt + 1) * NQ],
                                     start=(kc == 0), stop=(kc == KC - 1))
            for nt in range(2):
                nc.scalar.activation(
                    caVaug[:, sc, nt * 8:(nt + 1) * 8, 0:64],
                    pair[:, nt, :].rearrange("p (h w) -> p h w", w=64), AF.Copy)

        # ---- phase 6: cross-attention ----
        attention(caQT, caKT, caVaug, OT, masked=False)
        cakv.release()
        emat.release()

        # ---- phase 7: cross out-proj + residual (v^T in xTq) + LN2 ----
        co = ring.tile([P, KC, E], BF16, name="w")
        nc.sync.dma_start(co[:], wd["co"])
        btfp = tc.alloc_tile_pool(name="btf", bufs=1, side="left")
        bT_bf = btfp.tile([P, KC, NQ], BF16, name="bT_bf")
        for mp in range(4):
            pair = psc.tile([P, 2, NQ], F32, name="ps")
            for mi in range(2):
                m = 2 * mp + mi
                for kc in range(KC):
                    nc.tensor.matmul(pair[:, mi, :],
                                     co[:, kc, m * P:(m + 1) * P],
                                     OT[:, kc, :],
                                     start=(kc == 0), stop=(kc == KC - 1))
            for mi in range(2):
                m = 2 * mp + mi
                nc.vector.tensor_tensor(xTq[:, m, :], pair[:, mi, :],
                                        xTq[:, m, :], AluOpType.add)
        layernorm_bf(xTq, bT_bf)   # b^T in bf16; xTq stays un-normalized

        # ---- phase 8: FFN up (relu on DVE); prefetch FFN-down weights ----
        htp = tc.alloc_tile_pool(name="ht", bufs=1, side="right")
        HT = htp.tile([P, KH, NQ], BF16, name="HT")
        wp2 = tc.alloc_tile_pool(name="wp2", bufs=2, side="right")
        for g in range(4):
            w1g = ring.tile([P, KC, 1024], BF16, name="w")
            nc.sync.dma_start(w1g[:, :, 0:512], w1[g][:, :, 0:512])
            nc.sync.dma_start(w1g[:, :, 512:1024], w1[g][:, :, 512:1024])
            for half in range(2):
                pair = [psc.tile([P, 2, NQ], F32, name="ps") for _ in range(2)]
                for kc in range(KC):
                    for mi in range(4):
                        mh = 4 * half + mi
                        nc.tensor.matmul(pair[mi // 2][:, mi % 2, :],
                                         w1g[:, kc, mh * P:(mh + 1) * P],
                                         bT_bf[:, kc, :],
                                         start=(kc == 0), stop=(kc == KC - 1))
                for mi in range(4):
                    mh = 4 * half + mi
                    nc.vector.tensor_scalar_max(HT[:, g * 8 + mh, :],
                                                pair[mi // 2][:, mi % 2, :],
                                                0.0)

        # ---- phase 9: FFN down + residual + LN3 + store ----
        outp = tc.alloc_tile_pool(name="outp", bufs=1, side="left")
        outT_sb = outp.tile([P, KC, NQ], F32, name="outT_sb")
        sum_ps = pstat.tile([1, NQ], F32, name="s1")
        sq_ps = pstat.tile([1, NQ], F32, name="s2")
        for mp in range(4):
            w2g = wp2.tile([P, KH, 256], BF16, name="w2g")
            nc.sync.dma_start(w2g[:], w2[mp])
            pair = psc.tile([P, 2, NQ], F32, name="ps")
            for mi in range(2):
                m = 2 * mp + mi
                for kc2 in range(KH):
                    nc.tensor.matmul(pair[:, mi, :],
                                     w2g[:, kc2, mi * P:(mi + 1) * P],
                                     HT[:, kc2, :],
                                     start=(kc2 == 0), stop=(kc2 == KH - 1))
            for mi in range(2):
                m = 2 * mp + mi
                nc.vector.tensor_tensor(outT_sb[:, m, :], pair[:, mi, :],
                                        bT_bf[:, m, :], AluOpType.add)
                # incremental LN3 stats
                xb = sqp.tile([P, NQ], BF16, name="xb")
                sq = sqp.tile([P, NQ], BF16, name="sq")
                nc.scalar.activation(xb[:], outT_sb[:, m, :], AF.Copy)
                nc.vector.tensor_tensor(sq[:], xb[:], xb[:], AluOpType.mult)
                nc.tensor.matmul(sum_ps[:], ones_bf[:], xb[:],
                                 start=(m == 0), stop=(m == KC - 1))
                nc.tensor.matmul(sq_ps[:], ones_bf[:], sq[:],
                                 start=(m == 0), stop=(m == KC - 1))
        mu = rows.tile([1, NQ], F32, name="mu")
        ex2 = rows.tile([1, NQ], F32, name="ex2")
        var = rows.tile([1, NQ], F32, name="var")
        std = rows.tile([1, NQ], F32, name="std")
        rstd = rows.tile([1, NQ], F32, name="rstd")
        nc.scalar.activation(mu[:], sum_ps[:], AF.Copy, scale=1.0 / E)
        nc.scalar.activation(ex2[:], sq_ps[:], AF.Copy, scale=1.0 / E)
        nc.vector.tensor_tensor(var[:], mu[:], mu[:], AluOpType.mult)
        nc.vector.tensor_tensor(var[:], ex2[:], var[:], AluOpType.subtract)
        nc.scalar.activation(std[:], var[:], AF.Ln, bias=eps_t[0:1, :])
        nc.scalar.activation(rstd[:], std[:], AF.Exp, scale=-0.5)
        mub = rows.tile([1, NQ], BF16, name="mub")
        rsb = rows.tile([1, NQ], BF16, name="rsb")
        nc.scalar.activation(mub[:], mu[:], AF.Copy)
        nc.scalar.activation(rsb[:], rstd[:], AF.Copy)
        muB = pav.tile([P, NQ], F32, name="pav")
        nc.tensor.matmul(muB[:], ones_row[:], mub[:], start=True, stop=True)
        rsB = pav.tile([P, NQ], F32, name="pav")
        nc.tensor.matmul(rsB[:], ones_row[:], rsb[:], start=True, stop=True)
        for m in range(KC):
            nc.vector.tensor_tensor(outT_sb[:, m, :], outT_sb[:, m, :],
                                    muB[:], AluOpType.subtract)
            nc.vector.tensor_tensor(outT_sb[:, m, :], outT_sb[:, m, :],
                                    rsB[:], AluOpType.mult)
            nc.sync.dma_start(outT[:, m, :], outT_sb[:, m, :])

        # right side LIFO
        wp2.release()
        htp.release()
        ring.release()
        # left side LIFO
        outp.release()
        btfp.release()
        otp.release()
        xbqp.release()
        caqp.release()
        qtp.release()
        xtqp.release()
        sqp.release()
        bc.release()
        rows.release()
        pstat.release()
        pav.release()
        psc.release()
        const.release()

    nc.compile()
    return nc


def get_nc():
    global _NC_CACHE
    if _NC_CACHE is None:
        _NC_CACHE = build_nc()
    return _NC_CACHE


def _tile_pc(w):
    """[K*P, N] -> [P, K, N] contiguous chunk-tiled copy."""
    kp, n = w.shape
    return np.ascontiguousarray(
        w.reshape(kp // P, P, n).transpose(1, 0, 2))


def _qblocks(parity):
    return (0, 1, 6, 7) if parity == 0 else (2, 3, 4, 5)


def _qidx(parity):
    return np.concatenate([np.arange(128 * b, 128 * b + 128)
                           for b in _qblocks(parity)])


def make_in_maps(inputs):
    """Build per-core input maps from the full (unsharded) input dict."""
    bf = ml_dtypes.bfloat16
    tgt = np.asarray(inputs['target'], dtype=np.float32)
    enc = np.asarray(inputs['enc_src'], dtype=np.float32)
    wcast = {}
    for src_name, name in (('sa_wq', 'wq'), ('sa_wk', 'wk'), ('sa_wv', 'wv'),
                           ('sa_wo', 'wo'), ('ca_wq', 'cq'), ('ca_wk', 'ck'),
                           ('ca_wv', 'cv'), ('ca_wo', 'co')):
        wcast[name] = _tile_pc(np.asarray(inputs[src_name], np.float32)
                               .astype(bf))
    w1f = np.asarray(inputs['ff_w1'], np.float32).astype(bf)
    wcast['w1'] = np.ascontiguousarray(
        w1f.reshape(KC, P, 4, 1024).transpose(2, 1, 0, 3))
    w2f = np.asarray(inputs['ff_w2'], np.float32).astype(bf)
    wcast['w2'] = np.ascontiguousarray(
        w2f.reshape(KH, P, 4, 256).transpose(2, 1, 0, 3))
    sel = np.zeros((P, 256), np.float32)
    for r in range(4):
        sel[32 * r, 128 * (r // 2) + 64 * (r % 2):
            128 * (r // 2) + 64 * (r % 2) + 64] = 1.0
    wcast['sel4'] = np.ascontiguousarray(sel.astype(bf))
    in_maps = []
    k_loc = np.arange(P, dtype=np.int64)
    for c in range(8):
        b, par = c // 2, c % 2
        qi = _qidx(par)
        # mask: [:, ch, 0:256] covers G0 chunks 0-3 over queries qi[0:256];
        #       [:, ch, 256:512] covers G1 chunks 4-7 over queries qi[256:].
        m = np.zeros((P, 4, NQ), dtype=np.float32)
        for ch in range(4):
            kg0 = 128 * ch + k_loc          # G0 chunk = ch
            kg1 = 128 * (4 + ch) + k_loc    # G1 chunk = 4 + ch
            m[:, ch, 0:NG] = (kg0[:, None] >= qi[None, 0:NG])
            m[:, ch, NG:] = (kg1[:, None] >= qi[None, NG:])
        xTb = np.ascontiguousarray(tgt[b].T)
        xTbq = np.ascontiguousarray(xTb[:, qi])
        mm = {
            'xT': _tile_pc(xTb.astype(bf)),
            'xTq_bf': _tile_pc(xTbq.astype(bf)),
            'xTq_f': _tile_pc(xTbq),
            'eT': _tile_pc(enc[b].T.astype(bf)),
            'm01': np.ascontiguousarray(m.astype(bf)),
        }
        mm.update(wcast)
        in_maps.append(mm)
    return in_maps


def assemble(results):
    out = np.empty((4, T, E), dtype=np.float32)
    for c in range(8):
        b, par = c // 2, c % 2
        qi = _qidx(par)
        o = results[c]['outT']          # [P, KC, NQ]
        out[b, qi, :] = o.transpose(2, 1, 0).reshape(NQ, E)
    return out


def kernel(**inputs):
    nc = get_nc()
    in_maps = make_in_maps(inputs)
    res = run_bass_kernel_spmd(nc, in_maps, core_ids=list(range(8)))
    return assemble(res.results)
